# revision 1
# baseline (speedup 1.0000x reference)
"""2-layer GCN block (gcn_norm + 2x GCNConv/gelu + global mean pool) on
8 Trainium2 NeuronCores via Bass/Tile, SPMD with a 1D node partition.

kernel(**inputs) takes the FULL inputs of nn_GCNBlock_48747878809894 and
returns the full output (tuple of two (256, 64) float32 arrays).

Design:
  - norm = dis[src]*ew*dis[dst] factorized: each core scales its owned rows
    t = (h @ W) by dis before the halo exchange; dis[dst] is applied to
    aggregated 128-node windows afterwards. Self-loops are analytic:
    agg += t_own before the dis[dst] scale.
  - Halo exchange is S split AllGathers per layer (node-window groups), so
    collective latency overlaps the gather stream: gathers sweep src-group-
    major, consuming table_g right after AllGather_g lands while the next
    AllGather is still in flight on the collective cores.
  - Edges bucketed by (dst core, dst 128-node window, src group) on the
    host; each (window, group) padded to C_wg chunks of 128 edges (max over
    cores) so all 8 cores run a single SPMD program.
  - Per chunk: one indirect-DMA gather of 128 rows (256 B each) — the
    [128,1] index form is the only one the HW lowers correctly — then
    indicator matmuls accumulate
    psum[128 dst, 64] += eq[128e, 128d]^T @ (ew*gath)[128e, 64]
    per (window, group); group partials accumulate in SBUF.
  - Degrees via the same indicator matmuls against the edge-weight column;
    dis = sqrt(1/(deg+1)) (self-loop included analytically).
  - Global mean pool: indicator matmuls over two 128-graph-id windows
    accumulated in PSUM across all node windows; the host sums the 8
    per-core partials and divides by per-graph counts.
"""
import numpy as np

import concourse.bacc as bacc
import concourse.bass as bass
import concourse.mybir as mybir
import concourse.tile as tile
from concourse.masks import make_identity
from concourse.bass_utils import run_bass_kernel_spmd

F32 = mybir.dt.float32
I32 = mybir.dt.int32
AF = mybir.ActivationFunctionType
OP = mybir.AluOpType


class Cfg:
    def __init__(self, N=100000, E=1200000, D=64, G=256, K=8, S=2):
        self.N, self.E, self.D, self.G, self.K, self.S = N, E, D, G, K, S
        self.RPC = -(-N // K)            # rows per core
        self.W = -(-self.RPC // 128)     # node windows per core
        self.NPC = self.W * 128          # padded rows per core
        self.GW = -(-G // 128)           # graph-id windows
        self.Wg = -(-self.W // S)        # windows per group
        # windows of each group
        self.gwins = [list(range(g * self.Wg, min((g + 1) * self.Wg, self.W)))
                      for g in range(S)]
        self.Rg = [len(ws) * 128 for ws in self.gwins]   # rows/core/group


FULL = Cfg(S=1)


def prep_host(cfg, x, edge_index, edge_weight, batch):
    """Numpy-only sharding/index prep. Returns in-map arrays plus the
    per-(window, group) chunk counts (SPMD program shape)."""
    K, W, RPC, NPC, D, S = cfg.K, cfg.W, cfg.RPC, cfg.NPC, cfg.D, cfg.S
    Wg = cfg.Wg
    N = cfg.N
    src = np.asarray(edge_index[0], dtype=np.int64)
    dst = np.asarray(edge_index[1], dtype=np.int64)
    ewt = np.asarray(edge_weight, dtype=np.float32)
    batch = np.asarray(batch, dtype=np.int64)
    x = np.asarray(x, dtype=np.float32)

    # Renumber nodes so every 128-node window carries a near-equal edge
    # count (node order is internal): sort by in-degree, snake round-robin
    # over the K*W windows. Each window gets ceil/floor(N/(K*W)) nodes and
    # a balanced edge sum, so the per-window chunk count is minimal and
    # uniform across cores.
    NBINS = K * W
    deg_in = np.bincount(dst, minlength=N)
    nodeord = np.argsort(-deg_in, kind="stable")
    ranks = np.arange(N)
    stratum = ranks // NBINS
    posin = ranks % NBINS
    binid = np.where(stratum % 2 == 0, posin, NBINS - 1 - posin)
    perm_pad = np.empty(N, dtype=np.int64)       # node -> padded new row
    perm_pad[nodeord] = (binid // W) * NPC + (binid % W) * 128 + stratum
    row_node = np.full(K * NPC, -1, dtype=np.int64)  # padded row -> node
    row_node[perm_pad] = np.arange(N)

    pd = perm_pad[dst]
    ps = perm_pad[src]
    cd = pd // NPC                        # dst owner core
    ld = pd - cd * NPC                    # dst local (padded) row
    sc = ps // NPC                        # src owner core
    so = ps - sc * NPC                    # src local (padded) row
    sg = (so >> 7) // Wg                  # src group
    # row inside table_g: core block + (local row - group base)
    gbase = np.array([ws[0] * 128 for ws in cfg.gwins], dtype=np.int64)
    grows = np.array(cfg.Rg, dtype=np.int64)
    tab_row = sc * grows[sg] + (so - gbase[sg])

    bucket = (cd * W + (ld >> 7)) * S + sg          # (core, window, group)
    order = np.argsort(bucket, kind="stable")
    tab_s, ld_s, ew_s, b_s = tab_row[order], ld[order], ewt[order], bucket[order]

    bcounts = np.bincount(b_s, minlength=K * W * S).reshape(K, W * S)
    # per (window, group) chunk count: max over cores, at least 1
    Cwg = np.maximum(1, (bcounts.max(axis=0) + 127) // 128)     # [W*S]
    off = np.zeros(W * S + 1, dtype=np.int64)
    np.cumsum(Cwg, out=off[1:])
    CT = int(off[-1])

    starts = np.zeros(K * W * S, dtype=np.int64)
    np.cumsum(bcounts.ravel()[:-1], out=starts[1:])
    pos = np.arange(len(tab_s)) - starts[b_s]
    wg_of = b_s % (W * S)
    k_of = b_s // (W * S)
    flat = (k_of * CT + off[wg_of]) * 128 + pos

    srcp = np.zeros(K * CT * 128, dtype=np.int32)
    ewp = np.zeros(K * CT * 128, dtype=np.float32)
    dop = np.full(K * CT * 128, -1.0, dtype=np.float32)
    srcp[flat] = tab_s.astype(np.int32)
    ewp[flat] = ew_s
    dop[flat] = (ld_s & 127).astype(np.float32)

    def to_pm(a):     # [K*CT*128] -> [K, 128, CT]; slot index = c*128+p
        return a.reshape(K, CT, 128).transpose(0, 2, 1).copy()

    srcp, ewp, dop = to_pm(srcp), to_pm(ewp), to_pm(dop)

    real = row_node >= 0
    bp = np.where(real, batch[np.maximum(row_node, 0)], -1).astype(np.float32)
    batch_pm = bp.reshape(K, W, 128).transpose(0, 2, 1).copy()

    xp = np.where(real[:, None], x[np.maximum(row_node, 0)], 0.0)
    xp = xp.astype(np.float32).reshape(K, NPC, D)
    x_t = xp.transpose(0, 2, 1).copy()

    counts = np.bincount(batch, minlength=cfg.G).astype(np.float32)
    return x_t, srcp, ewp, dop, batch_pm, counts, tuple(int(c) for c in Cwg)


def build_nc(cfg, Cwg, debug=False):
    K, W, NPC, D, GW, S = cfg.K, cfg.W, cfg.NPC, cfg.D, cfg.GW, cfg.S
    off = [0]
    for c in Cwg:
        off.append(off[-1] + c)
    CT = off[-1]

    def crange(w, g):             # chunk-column range of (window, group)
        i = w * S + g
        return off[i], off[i + 1]

    # chunk range of a whole window (all groups contiguous)
    def wrange(w):
        return off[w * S], off[(w + 1) * S]

    Cmax_wg = max(Cwg)
    Cmax_w = max(wrange(w)[1] - wrange(w)[0] for w in range(W))

    nc = bacc.Bacc("TRN2", target_bir_lowering=False, debug=debug)

    x_t_d = nc.dram_tensor("x_t", [D, NPC], F32, kind="ExternalInput")
    src_d = nc.dram_tensor("srcidx", [128, CT], I32, kind="ExternalInput")
    ew_d = nc.dram_tensor("ew", [128, CT], F32, kind="ExternalInput")
    do_d = nc.dram_tensor("dstoff", [128, CT], F32, kind="ExternalInput")
    bat_d = nc.dram_tensor("batch_pm", [128, W], F32, kind="ExternalInput")
    w0_d = nc.dram_tensor("w0", [D, D], F32, kind="ExternalInput")
    w1_d = nc.dram_tensor("w1", [D, D], F32, kind="ExternalInput")
    b0_d = nc.dram_tensor("b0b", [128, D], F32, kind="ExternalInput")
    b1_d = nc.dram_tensor("b1b", [128, D], F32, kind="ExternalInput")
    iota_d = nc.dram_tensor("iota", [128, 128], F32, kind="ExternalInput")
    iotag_d = [nc.dram_tensor(f"iotag{gw}", [128, 128], F32,
                              kind="ExternalInput") for gw in range(GW)]
    pool_out = [nc.dram_tensor(f"pool{L}", [GW * 128, D], F32,
                               kind="ExternalOutput") for L in (0, 1)]

    rg = [list(range(K))]

    with tile.TileContext(nc) as tc:
        with tc.tile_pool(name="const", bufs=1) as cpool, \
             tc.tile_pool(name="state", bufs=1) as spool, \
             tc.tile_pool(name="dram", bufs=1, space="DRAM") as dpool, \
             tc.tile_pool(name="eqa_p", bufs=2) as eqa_p, \
             tc.tile_pool(name="gath_p", bufs=3) as gath_p, \
             tc.tile_pool(name="gsc_p", bufs=2) as gsc_p, \
             tc.tile_pool(name="small_p", bufs=3) as small_p, \
             tc.tile_pool(name="xT_p", bufs=2) as xT_p, \
             tc.tile_pool(name="ps_misc", bufs=2, space="PSUM") as ps_misc, \
             tc.tile_pool(name="ps_t", bufs=2, space="PSUM") as ps_t, \
             tc.tile_pool(name="ps_agg", bufs=2, space="PSUM") as ps_agg, \
             tc.tile_pool(name="ps_pool", bufs=GW, space="PSUM") as ps_pool:

            iota_t = cpool.tile([128, 128], F32, name="iota_t")
            nc.sync.dma_start(iota_t[:], iota_d[:])
            iotag_t = []
            for gw in range(GW):
                tgi = cpool.tile([128, 128], F32, name=f"iotag_t{gw}")
                nc.sync.dma_start(tgi[:], iotag_d[gw][:])
                iotag_t.append(tgi)
            wt = []
            for L, wd in enumerate((w0_d, w1_d)):
                wti = cpool.tile([D, D], F32, name=f"w_t{L}")
                nc.sync.dma_start(wti[:], wd[:])
                wt.append(wti)
            bt = []
            for L, bd in enumerate((b0_d, b1_d)):
                bti = cpool.tile([128, D], F32, name=f"b_t{L}")
                nc.sync.dma_start(bti[:], bd[:])
                bt.append(bti)
            ident = cpool.tile([128, 128], F32, name="ident")
            make_identity(nc, ident[:])

            src_all = spool.tile([128, CT], I32, name="src_all")
            nc.sync.dma_start(src_all[:], src_d[:])
            ew_all = spool.tile([128, CT], F32, name="ew_all")
            nc.sync.dma_start(ew_all[:], ew_d[:])
            do_all = spool.tile([128, CT], F32, name="do_all")
            nc.sync.dma_start(do_all[:], do_d[:])
            bat_all = spool.tile([128, W], F32, name="bat_all")
            nc.sync.dma_start(bat_all[:], bat_d[:])
            dis_sb = spool.tile([128, W], F32, name="dis_sb")
            t_own = [spool.tile([128, W * D], F32, name=f"t_own{L}")
                     for L in (0, 1)]
            g_all = [spool.tile([128, W * D], F32, name=f"g_all{L}")
                     for L in (0, 1)]
            agg_sb = spool.tile([128, W * D], F32, name="agg_sb")

            ag_in = [[dpool.tile([cfg.Rg[g], D], F32, name=f"ag_in{L}_{g}")
                      for g in range(S)] for L in (0, 1)]
            t_full = [[dpool.tile([K * cfg.Rg[g], D], F32,
                                  name=f"t_full{L}_{g}", addr_space="Shared")
                       for g in range(S)] for L in (0, 1)]

            dis_w = [None] * W

            def phase_a(w):
                lo, hi = wrange(w)
                C = hi - lo
                eqa = eqa_p.tile([128, Cmax_w, 128], F32, name="eqa")
                eng = nc.gpsimd if w % 3 == 2 else nc.vector
                for c in range(C):
                    eng.tensor_scalar(
                        eqa[:, c, :], iota_t[:],
                        do_all[:, lo + c: lo + c + 1], None, OP.is_equal)
                degp = ps_misc.tile([128, 1], F32, name="degp", tag="misc", space="PSUM")
                for c in range(C):
                    nc.tensor.matmul(
                        degp[:], lhsT=eqa[:, c, :],
                        rhs=ew_all[:, lo + c: lo + c + 1],
                        start=(c == 0), stop=(c == C - 1))
                degs = small_p.tile([128, 1], F32, name="degs")
                nc.scalar.add(degs[:], degp[:], 1.0)
                rec = small_p.tile([128, 1], F32, name="rec")
                nc.vector.reciprocal(rec[:], degs[:])
                nc.scalar.sqrt(dis_sb[:, w:w + 1], rec[:])
                dis_w[w] = dis_sb[:, w:w + 1]

            def b1(L, w):
                """t'_L(w) = dis(w) * (h_L(w) @ W_L) into t_own; for L=1
                also store to the AllGather input."""
                if L == 0:
                    xT = xT_p.tile([D, 128], F32, name="xT")
                    nc.sync.dma_start(xT[:],
                                      x_t_d[:, w * 128:(w + 1) * 128])
                else:
                    trp = ps_misc.tile([D, 128], F32, name="trp", tag="misc",
                                       space="PSUM")
                    nc.tensor.transpose(
                        trp[:], g_all[0][:, w * D:(w + 1) * D], ident[:])
                    xT = xT_p.tile([D, 128], F32, name="xT")
                    nc.scalar.copy(xT[:], trp[:])
                tp = ps_t.tile([128, D], F32, name="tp", space="PSUM")
                nc.tensor.matmul(tp[:], lhsT=xT[:], rhs=wt[L][:],
                                 start=True, stop=True)
                ts = t_own[L][:, w * D:(w + 1) * D]
                nc.scalar.mul(ts, tp[:], dis_w[w])
                g = min(w // cfg.Wg, S - 1)
                base = cfg.gwins[g][0] * 128
                nc.sync.dma_start(
                    ag_in[L][g][w * 128 - base: (w + 1) * 128 - base, :], ts)

            def allgather(L, g):
                nc.gpsimd.collective_compute(
                    "AllGather", OP.bypass,
                    ins=[ag_in[L][g].opt()], outs=[t_full[L][g].opt()],
                    replica_groups=rg)

            def b3_group(L, w, g, pps):
                """Gather+aggregate group-g chunks of window w into
                psum, then fold into agg_sb; on the last group run the
                post-ops (self-loop, dis, bias, gelu, pooling)."""
                lo, hi = crange(w, g)
                C = hi - lo
                gath = gath_p.tile([128, Cmax_wg * D], F32, name="gath")
                for c in range(C):
                    col = lo + c
                    nc.gpsimd.indirect_dma_start(
                        out=gath[:, c * D:(c + 1) * D], out_offset=None,
                        in_=t_full[L][g][:],
                        in_offset=bass.IndirectOffsetOnAxis(
                            ap=src_all[:, col:col + 1], axis=0))
                gsc = gsc_p.tile([128, Cmax_wg, D], F32, name="gsc")
                for c in range(C):
                    nc.vector.tensor_scalar(
                        gsc[:, c, :], gath[:, c * D:(c + 1) * D],
                        ew_all[:, lo + c: lo + c + 1], None, OP.mult)
                eqa = eqa_p.tile([128, Cmax_w, 128], F32, name="eqa")
                for c in range(C):
                    nc.vector.tensor_scalar(
                        eqa[:, c, :], iota_t[:],
                        do_all[:, lo + c: lo + c + 1], None, OP.is_equal)
                aggp = ps_agg.tile([128, D], F32, name="aggp", space="PSUM")
                for c in range(C):
                    nc.tensor.matmul(aggp[:], lhsT=eqa[:, c, :],
                                     rhs=gsc[:, c, :],
                                     start=(c == 0), stop=(c == C - 1))
                dsl = slice(w * D, (w + 1) * D)
                if g == 0 and S > 1:
                    nc.vector.tensor_copy(agg_sb[:, dsl], aggp[:])
                    return
                if g < S - 1:
                    nc.vector.tensor_tensor(out=agg_sb[:, dsl],
                                            in0=agg_sb[:, dsl],
                                            in1=aggp[:], op=OP.add)
                    return
                # last group: fold psum + (earlier groups) + self-loop
                pre = small_p.tile([128, D], F32, name="pre")
                if S > 1:
                    nc.vector.tensor_tensor(out=pre[:], in0=aggp[:],
                                            in1=agg_sb[:, dsl], op=OP.add)
                    nc.vector.tensor_tensor(out=pre[:], in0=pre[:],
                                            in1=t_own[L][:, dsl], op=OP.add)
                else:
                    nc.vector.tensor_tensor(out=pre[:], in0=aggp[:],
                                            in1=t_own[L][:, dsl], op=OP.add)
                scb = small_p.tile([128, D], F32, name="scb")
                nc.scalar.mul(scb[:], pre[:], dis_w[w])
                scb2 = small_p.tile([128, D], F32, name="scb2")
                nc.vector.tensor_tensor(out=scb2[:], in0=scb[:],
                                        in1=bt[L][:], op=OP.add)
                gout = g_all[L][:, dsl]
                nc.scalar.activation(gout, scb2[:], AF.Gelu)
                for gw in range(GW):
                    eqp = small_p.tile([128, 128], F32, name=f"eqp{gw}")
                    nc.vector.tensor_scalar(eqp[:], iotag_t[gw][:],
                                            bat_all[:, w:w + 1], None,
                                            OP.is_equal)
                    nc.tensor.matmul(pps[gw][:], lhsT=eqp[:], rhs=gout,
                                     start=(w == 0), stop=(w == W - 1))

            # ---- program ----
            # phase A + B1(L0), grouped; AllGather_g(L0) after each group
            for g in range(S):
                for w in cfg.gwins[g]:
                    phase_a(w)
                    b1(0, w)
                allgather(0, g)

            # B3(L0) sweep, src-group-major; B1(L1) + AllGather(L1) chunks
            # fire as soon as their windows complete in the last sweep
            pps0 = [ps_pool.tile([128, D], F32, name=f"pps0_{gw}",
                                 tag="pps", space="PSUM") for gw in range(GW)]
            for g in range(S):
                last = (g == S - 1)
                for w in range(W):
                    b3_group(0, w, g, pps0)
                    if last:
                        b1(1, w)
                        for gg in range(S):
                            if w == cfg.gwins[gg][-1]:
                                allgather(1, gg)
            for gw in range(GW):
                pok = small_p.tile([128, D], F32, name=f"pok{gw}")
                nc.scalar.copy(pok[:], pps0[gw][:])
                nc.sync.dma_start(pool_out[0][gw * 128:(gw + 1) * 128, :],
                                  pok[:])

            # B3(L1) sweep
            pps1 = [ps_pool.tile([128, D], F32, name=f"pps1_{gw}",
                                 tag="pps", space="PSUM") for gw in range(GW)]
            for g in range(S):
                for w in range(W):
                    b3_group(1, w, g, pps1)
            for gw in range(GW):
                pok = small_p.tile([128, D], F32, name=f"pok{gw}")
                nc.scalar.copy(pok[:], pps1[gw][:])
                nc.sync.dma_start(pool_out[1][gw * 128:(gw + 1) * 128, :],
                                  pok[:])

    nc.finalize()
    return nc


_NC_CACHE = {}


def get_nc(cfg, Cwg):
    key = (cfg.N, cfg.E, cfg.G, cfg.K, cfg.S, Cwg)
    if key not in _NC_CACHE:
        _NC_CACHE[key] = build_nc(cfg, Cwg)
    return _NC_CACHE[key]


def make_in_maps(cfg, x_t, srcp, ewp, dop, batch_pm, W0, b0, W1, b1):
    D = cfg.D
    b0b = np.ascontiguousarray(
        np.broadcast_to(np.asarray(b0, np.float32), (128, D)))
    b1b = np.ascontiguousarray(
        np.broadcast_to(np.asarray(b1, np.float32), (128, D)))
    iota = np.ascontiguousarray(
        np.broadcast_to(np.arange(128, dtype=np.float32), (128, 128)))
    maps = []
    for k in range(cfg.K):
        m = {
            "x_t": x_t[k], "srcidx": srcp[k], "ew": ewp[k], "dstoff": dop[k],
            "batch_pm": batch_pm[k],
            "w0": np.asarray(W0, np.float32), "w1": np.asarray(W1, np.float32),
            "b0b": b0b, "b1b": b1b, "iota": iota,
        }
        for gw in range(cfg.GW):
            m[f"iotag{gw}"] = iota + gw * 128
        maps.append(m)
    return maps


def postprocess(cfg, results, counts):
    outs = []
    denom = np.maximum(counts, 1.0).astype(np.float32)
    for L in (0, 1):
        tot = np.zeros((cfg.GW * 128, cfg.D), dtype=np.float32)
        for k in range(cfg.K):
            tot += results[k][f"pool{L}"]
        outs.append((tot[: cfg.G] / denom[:, None]).astype(np.float32))
    return tuple(outs)


def kernel(x, edge_index, edge_weight, batch, W0, b0, W1, b1):
    cfg = FULL
    x_t, srcp, ewp, dop, batch_pm, counts, Cwg = prep_host(
        cfg, x, edge_index, edge_weight, batch)
    nc = get_nc(cfg, Cwg)
    in_maps = make_in_maps(cfg, x_t, srcp, ewp, dop, batch_pm, W0, b0, W1, b1)
    res = run_bass_kernel_spmd(nc, in_maps, list(range(cfg.K)))
    return postprocess(cfg, res.results, counts)



# revision 29
# speedup vs baseline: 2.8254x; 2.8254x over previous
"""2-layer GCN block (gcn_norm + 2x GCNConv/gelu + global mean pool) on
8 Trainium2 NeuronCores via Bass/Tile, SPMD, src-partitioned.

Design (v10):
  - Nodes renumbered and partitioned by OWNER core (12544 padded rows/core,
    98 windows of 128, 49 window-PAIRS/core, 392 global pairs). The GCN
    normalization (deg, dis, per-edge norm ewn = dis[s]*ew*dis[d], and the
    self-loop coefficient dis^2) is precomputed on the host like the
    per-graph node counts.
  - Each layer, each core computes t = h_own @ W for its OWN rows only
    (bf16), writes the 12544x(64+64pad) bf16 table to DRAM (256B rows).
  - Edges are processed by their SRC owner. One dma_gather per ~48-chunk
    batch pulls the per-edge src rows (128 rows/chunk) from the local
    table. Per chunk, TWO matmuls with a host-streamed fp8 one-hot matrix
    (dst-row-in-pair one-hot, scaled by ewn: the gather's edge scaling and
    the scatter indicator fused into static data) accumulate the chunk's
    messages into a [128, 2x64] PSUM tile per dst window-pair.
  - Completed pairs are copied PSUM->SBUF (bf16, DVE/Act alternating) and
    written to a [K*12544, 64] bf16 partial buffer; ONE ReduceScatter per
    layer (output only 1.6MB -> ~55us vs 284us for an AllGather) gives
    each core the full aggregation for its own rows.
  - Post: h = gelu(agg + dis2*t_own + b) batched over all windows; global
    mean pooling via host-streamed fp8 graph-indicator matmuls into PSUM;
    host divides by counts and sums the 8 per-core partials.
  - Layer 1 lhsT comes from ONE dma_start_transpose of h0 (no PE
    transposes).
"""
import numpy as np
import ml_dtypes

import concourse.bacc as bacc
import concourse.bass as bass
import concourse.mybir as mybir
import concourse.tile as tile
from concourse.bass_utils import run_bass_kernel_spmd
from concourse.library_config import mlp

F32 = mybir.dt.float32
BF16 = mybir.dt.bfloat16
FP8 = mybir.dt.float8e4
I16 = mybir.dt.int16
AF = mybir.ActivationFunctionType
OP = mybir.AluOpType

NPF8 = ml_dtypes.float8_e4m3fn
NPBF = ml_dtypes.bfloat16


class Cfg:
    def __init__(self, N=100000, E=1200000, D=64, G=256, K=8):
        self.N, self.E, self.D, self.G, self.K = N, E, D, G, K
        # 100 windows/core: mean edges per (core, window-pair) bucket is
        # E/K/400 = 375, comfortably under the 384 = 3*128 chunk boundary,
        # so the host balancer can hold nearly every bucket to 3 chunks.
        self.W = 100                  # windows per core
        self.NPC = self.W * 128       # padded rows per core (12800)
        self.PPC = self.W // 2        # pairs per core (50)
        self.NP = self.K * self.PPC   # global pairs (400)
        self.GW = -(-G // 128)        # graph-id windows (2)
        self.BCH = 48                 # chunks per gather/eqa batch
        self.SGRP = 10                # pairs per partial-write group


FULL = Cfg()


def _assign_pairs(vecs, npairs, cap):
    """Greedy multi-dim balance: assign each node (row of vecs [n, K]) to a
    pair, minimizing the resulting max per-src-core in-edge count, capacity
    `cap` nodes per pair. Returns pair index per node."""
    n, kk = vecs.shape
    order = np.argsort(-vecs.sum(1), kind="stable")
    S = np.zeros((npairs, kk), dtype=np.int64)
    cnt = np.zeros(npairs, dtype=np.int64)
    out = np.empty(n, dtype=np.int64)
    for i in order:
        v = vecs[i]
        score = (S + v).max(axis=1).astype(np.float64)
        score[cnt >= cap] = np.inf
        # tie-break on emptiest bin to keep counts even
        b = np.argmin(score + cnt * 1e-6)
        out[i] = b
        S[b] += v
        cnt[b] += 1
    return out


def prep_host(cfg, x, edge_index, edge_weight, batch):
    N, E, D, K, W = cfg.N, cfg.E, cfg.D, cfg.K, cfg.W
    NPC, PPC, NP = cfg.NPC, cfg.PPC, cfg.NP
    src = np.asarray(edge_index[0], dtype=np.int64)
    dst = np.asarray(edge_index[1], dtype=np.int64)
    ew = np.asarray(edge_weight, dtype=np.float64)
    batch = np.asarray(batch, dtype=np.int64)
    x = np.asarray(x, dtype=np.float32)

    # ---- gcn_norm on host (graph preprocessing, like the pool counts) ----
    deg = np.bincount(dst, weights=ew, minlength=N) + 1.0
    dis = 1.0 / np.sqrt(deg)
    ewn = (dis[src] * ew * dis[dst]).astype(np.float32)
    dis2 = (dis * dis).astype(np.float32)

    # ---- node -> (core, local row) numbering ----
    outdeg = np.bincount(src, minlength=N)
    order = np.argsort(-outdeg, kind="stable")
    ranks = np.arange(N)
    stratum, posin = ranks // K, ranks % K
    core_rank = np.where(stratum % 2 == 0, posin, K - 1 - posin)
    core_of = np.empty(N, dtype=np.int64)
    core_of[order] = core_rank

    # in-edge count of each node split by src core
    src_core = core_of[src]
    vq = np.bincount(dst * K + src_core, minlength=N * K).reshape(N, K)

    dloc = np.empty(N, dtype=np.int64)     # local row within the core
    for c in range(K):
        nodes = np.nonzero(core_of == c)[0]
        pair = _assign_pairs(vq[nodes], PPC, 256)
        posin_pair = np.zeros(len(nodes), dtype=np.int64)
        cnts = np.zeros(PPC, dtype=np.int64)
        for ii, p in enumerate(pair):
            posin_pair[ii] = cnts[p]
            cnts[p] += 1
        dloc[nodes] = pair * 256 + posin_pair

    grow = core_of * NPC + dloc            # node -> global padded row
    row_node = np.full(K * NPC, -1, dtype=np.int64)
    row_node[grow] = np.arange(N)

    # ---- edge slot schedule (shared across cores) ----
    e_core = src_core                                   # processing core
    e_pair = (core_of[dst] * PPC) + (dloc[dst] >> 8)    # global dst pair
    cntkp = np.bincount(e_core * NP + e_pair,
                        minlength=K * NP).reshape(K, NP)
    Cs = np.maximum(1, (cntkp.max(axis=0) + 127) // 128)        # [NP]
    off = np.zeros(NP + 1, dtype=np.int64)
    np.cumsum(Cs, out=off[1:])
    CT = int(off[-1])
    SLOTS = CT * 128

    bucket = e_core * NP + e_pair
    eorder = np.argsort(bucket, kind="stable")
    starts = np.zeros(K * NP, dtype=np.int64)
    np.cumsum(cntkp.ravel()[:-1], out=starts[1:])
    pos = np.arange(E) - starts[bucket[eorder]]
    slot = off[e_pair[eorder]] * 128 + pos              # slot within core
    es, ed, ewn_s = src[eorder], dst[eorder], ewn[eorder]
    ecore_s = e_core[eorder]

    # gather table row of src: r = p*W + w  (partition-major)
    sw, sp = dloc[es] >> 7, dloc[es] & 127
    tabrow = sp * W + sw
    # dst row-in-pair (0..255)
    jrow = dloc[ed] & 255

    # per-core streams; one-hot eqa holds EXACT 1.0 in fp8, the edge norm
    # rides in a separate f32 sidecar applied on the DVE
    idxw = np.zeros((K, 128, SLOTS // 16), dtype=np.int16)
    eqa = np.zeros((K, 128, CT * 256), dtype=NPF8)
    ewn_pm = np.zeros((K, 128, CT), dtype=np.float32)
    for c in range(K):
        m = ecore_s == c
        s_c, tr_c, j_c, wv_c = slot[m], tabrow[m], jrow[m], ewn_s[m]
        idxf = np.zeros(SLOTS, dtype=np.int16)
        idxf[s_c] = tr_c.astype(np.int16)
        iw = idxf.reshape(-1, 16).T                     # [16, SLOTS/16]
        idxw[c] = np.tile(iw, (8, 1))
        chunk = s_c >> 7
        erow = s_c & 127
        flat = erow * (CT * 256) + chunk * 256 + j_c
        ef = eqa[c].reshape(-1)
        ef[flat] = np.float32(1.0).astype(NPF8)
        ewn_pm[c][erow, chunk] = wv_c

    # ---- per-core node-indexed tensors ----
    real = row_node >= 0
    nid = np.maximum(row_node, 0)
    xw = np.where(real[:, None], x[nid], 0.0).astype(NPBF)   # [K*NPC, D]
    # x_t: [K, 64, NPC] columns in w-major local order (col = dloc)
    x_t = xw.reshape(K, NPC, D).transpose(0, 2, 1).copy()

    d2 = np.where(real, dis2[nid], 0.0).astype(np.float32)
    # dis2_pm [K, 128, W]: [p, w] = dis2 of dloc w*128+p
    dis2_pm = d2.reshape(K, W, 128).transpose(0, 2, 1).copy()

    bat = np.where(real, batch[nid], -1)
    eqp = np.zeros((K, 128, W * 256), dtype=NPF8)
    bkw = bat.reshape(K, W, 128)
    for c in range(K):
        p_i, w_i = np.meshgrid(np.arange(128), np.arange(W), indexing="ij")
        g = bkw[c].T                                    # [128, W]
        valid = g >= 0
        flat = (p_i * (W * 256) + w_i * 256 + g)[valid]
        ef = eqp[c].reshape(-1)
        ef[flat] = np.float32(1.0).astype(NPF8)

    counts = np.bincount(batch, minlength=cfg.G).astype(np.float32)
    data = {"x_t": x_t, "idxw": idxw, "eqa": eqa, "ewn": ewn_pm,
            "eqp": eqp, "dis2": dis2_pm}
    return data, counts, tuple(int(v) for v in Cs)


def build_nc(cfg, Cs, debug=False, act=AF.Gelu):
    K, W, NPC, D, GW = cfg.K, cfg.W, cfg.NPC, cfg.D, cfg.GW
    NP, PPC, BCH, SGRP = cfg.NP, cfg.PPC, cfg.BCH, cfg.SGRP
    off = [0]
    for c in Cs:
        off.append(off[-1] + c)
    CT = off[-1]
    # chunk -> pair, first/last flags
    cpair = np.empty(CT, dtype=np.int64)
    cfirst = np.zeros(CT, dtype=bool)
    clast = np.zeros(CT, dtype=bool)
    for p in range(NP):
        cpair[off[p]:off[p + 1]] = p
        cfirst[off[p]] = True
        clast[off[p + 1] - 1] = True
    batches = [(lo, min(lo + BCH, CT)) for lo in range(0, CT, BCH)]

    nc = bacc.Bacc("TRN2", target_bir_lowering=False, debug=debug)

    xt_d = nc.dram_tensor("x_t", [D, NPC], BF16, kind="ExternalInput")
    idx_d = nc.dram_tensor("idxw", [128, CT * 8], I16, kind="ExternalInput")
    eqa_d = nc.dram_tensor("eqa", [128, CT * 256], FP8, kind="ExternalInput")
    ewn_d = nc.dram_tensor("ewn", [128, CT], F32, kind="ExternalInput")
    eqp_d = nc.dram_tensor("eqp", [128, W * 256], FP8, kind="ExternalInput")
    dis2_d = nc.dram_tensor("dis2", [128, W], F32, kind="ExternalInput")
    w_d = [nc.dram_tensor(f"w{L}", [D, D], BF16, kind="ExternalInput")
           for L in (0, 1)]
    b_d = [nc.dram_tensor(f"b{L}b", [128, D], BF16, kind="ExternalInput")
           for L in (0, 1)]
    pool_out = [nc.dram_tensor(f"pool{L}", [GW * 128, D], F32,
                               kind="ExternalOutput") for L in (0, 1)]

    rg = [list(range(K))]

    with tile.TileContext(nc) as tc:
        with tc.tile_pool(name="const", bufs=1) as cpool, \
             tc.tile_pool(name="state", bufs=1) as spool, \
             tc.tile_pool(name="lhsT_p", bufs=1) as lhsT_p, \
             tc.tile_pool(name="dram", bufs=1, space="DRAM") as dpool, \
             tc.tile_pool(name="gath_p", bufs=2) as gath_p, \
             tc.tile_pool(name="eqa_p", bufs=2) as eqa_p, \
             tc.tile_pool(name="stage_p", bufs=3) as stage_p, \
             tc.tile_pool(name="ps_t", bufs=2, space="PSUM") as ps_t, \
             tc.tile_pool(name="ps_pair", bufs=3, space="PSUM") as ps_pair, \
             tc.tile_pool(name="ps_pool", bufs=2, space="PSUM") as ps_pool:

            nc.gpsimd.load_library(mlp)

            # consts
            wt = []
            for L in (0, 1):
                t = cpool.tile([D, D], BF16, name=f"wt{L}")
                nc.sync.dma_start(t[:], w_d[L][:])
                wt.append(t)
            bt = []
            for L in (0, 1):
                t = cpool.tile([128, D], BF16, name=f"bt{L}")
                nc.sync.dma_start(t[:], b_d[L][:])
                bt.append(t)
            dis2_sb = cpool.tile([128, W], F32, name="dis2_sb")
            nc.sync.dma_start(dis2_sb[:], dis2_d[:])
            eqp_sb = cpool.tile([128, W * 256], FP8, name="eqp_sb")
            nc.sync.dma_start(eqp_sb[:], eqp_d[:])
            idx_sb = cpool.tile([128, CT * 8], I16, name="idx_sb")
            nc.sync.dma_start(idx_sb[:], idx_d[:])
            ewn_sb = cpool.tile([128, CT], F32, name="ewn_sb")
            nc.sync.dma_start(ewn_sb[:], ewn_d[:])

            t_sb = spool.tile([128, W * 128], BF16, name="t_sb")
            nc.vector.memset(t_sb[:], 0)      # pad halves stay zero
            h_sb = spool.tile([128, W * D], BF16, name="h_sb")   # scratch
            rs_sb = spool.tile([128, W * D], BF16, name="rs_sb")  # rs, then h

            table_d = dpool.tile([NPC, 128], BF16, name="table")
            parts_d = dpool.tile([K * NPC, D], BF16, name="parts")
            rs_out_d = dpool.tile([NPC, D], BF16, name="rs_out")
            h0_d = dpool.tile([NPC, D], BF16, name="h0")

            xt_sb = lhsT_p.tile([D, NPC], BF16, name="xt_sb")
            nc.sync.dma_start(xt_sb[:], xt_d[:])

            # partials DRAM view: [k][p][q=(pair_local*2+w01)][64]
            parts_v = parts_d[:].rearrange("(k p q) e -> k p (q e)",
                                           k=K, p=128)
            # rs_out rows r=p*W+w -> per-partition contiguous
            rs_v = rs_out_d[:].rearrange("(p r) e -> p (r e)", p=128)
            # h0 rows d = w*128+p
            h0_v = h0_d[:].rearrange("(w p) e -> p w e", p=128)
            # table rows r = p*W+w
            tab_v = table_d[:].rearrange("(p r) e -> p (r e)", p=128)

            t3 = t_sb[:].rearrange("p (w e) -> p w e", e=128)
            h3 = h_sb[:].rearrange("p (w e) -> p w e", e=D)
            rs3 = rs_sb[:].rearrange("p (w e) -> p w e", e=D)

            hT_sb = None

            def t_phase(L, lhsT):
                # t = h @ W  (8 windows per PSUM bank)
                for wb in range(0, W, 8):
                    nwin = min(8, W - wb)
                    pt = ps_t.tile([128, nwin * D], F32, name="pt",
                                   space="PSUM")
                    for i in range(nwin):
                        w = wb + i
                        # one zero-region: first mm starts, last stops
                        nc.tensor.matmul(
                            pt[:, i * D:(i + 1) * D],
                            lhsT=lhsT[:, w * 128:(w + 1) * 128],
                            rhs=wt[L][:], start=(i == 0),
                            stop=(i == nwin - 1))
                    dst = t3[:, wb:wb + nwin, 0:D]
                    src = pt[:].rearrange("p (w e) -> p w e", e=D)
                    eng = nc.vector if (wb // 8) % 2 == 0 else nc.scalar
                    if eng is nc.vector:
                        eng.tensor_copy(dst, src)
                    else:
                        eng.copy(dst, src)
                nc.sync.dma_start(tab_v, t_sb[:])

            def stream(L):
                pair_ps = {}
                for (lo, hi) in batches:
                    nch = hi - lo
                    gath = gath_p.tile([128, BCH * 128], BF16, name="gath")
                    g3 = gath[:].rearrange("p (c e) -> p c e", e=128)
                    nc.gpsimd.dma_gather(
                        g3[:, 0:nch, :],
                        table_d[:], idx_sb[:, lo * 8:hi * 8],
                        nch * 128, nch * 128, 128, single_packet=False)
                    # scale the gathered rows by the f32 edge norm in place
                    wb_ = ewn_sb[:, lo:hi].unsqueeze(2).broadcast_to(
                        (128, nch, D))
                    nc.vector.tensor_tensor(out=g3[:, 0:nch, 0:D],
                                            in0=g3[:, 0:nch, 0:D],
                                            in1=wb_, op=OP.mult)
                    eqa_t = eqa_p.tile([128, BCH * 256], FP8, name="eqa_t")
                    nc.sync.dma_start(eqa_t[:, 0:nch * 256],
                                      eqa_d[:, lo * 256:hi * 256])
                    for c in range(lo, hi):
                        pr = int(cpair[c])
                        if cfirst[c]:
                            pair_ps[pr] = ps_pair.tile(
                                [128, 2 * D], F32, name="pp",
                                space="PSUM")
                        pp = pair_ps[pr]
                        cb = (c - lo) * 256
                        rhs = gath[:, (c - lo) * 128:(c - lo) * 128 + D]
                        # both windows share one psum zero-region: only the
                        # pair's very first mm starts it, the last stops it
                        nc.tensor.matmul(pp[:, 0:D],
                                         lhsT=eqa_t[:, cb:cb + 128],
                                         rhs=rhs, start=bool(cfirst[c]),
                                         stop=False)
                        nc.tensor.matmul(pp[:, D:2 * D],
                                         lhsT=eqa_t[:, cb + 128:cb + 256],
                                         rhs=rhs, start=False,
                                         stop=bool(clast[c]))
                        if clast[c]:
                            g = pr // SGRP
                            gslot = pr % SGRP
                            if gslot == 0:
                                stage_t = stage_p.tile(
                                    [128, SGRP * 2 * D], BF16, name="stage")
                                pair_ps["stage"] = stage_t
                            stage_t = pair_ps["stage"]
                            dstp = stage_t[:, gslot * 2 * D:(gslot + 1) * 2 * D]
                            if pr % 2 == 0:
                                nc.vector.tensor_copy(dstp, pp[:])
                            else:
                                nc.scalar.copy(dstp, pp[:])
                            del pair_ps[pr]
                            if gslot == SGRP - 1:
                                kd = pr // PPC
                                pl0 = (g % (PPC // SGRP)) * SGRP
                                nc.sync.dma_start(
                                    parts_v[kd][:, pl0 * 2 * D:
                                                (pl0 + SGRP) * 2 * D],
                                    stage_t[:])

            def post(L):
                nonlocal hT_sb
                nc.gpsimd.collective_compute(
                    "ReduceScatter", OP.add,
                    ins=[parts_d[:]], outs=[rs_out_d[:]], replica_groups=rg)
                nc.sync.dma_start(rs_sb[:], rs_v)
                # h = gelu(rs + dis2 * t + b); h_sb is scratch, the final
                # activations land in rs_sb (rs no longer needed then).
                d2b = dis2_sb[:].unsqueeze(2).broadcast_to((128, W, D))
                nc.vector.tensor_tensor(out=h3, in0=t3[:, :, 0:D],
                                        in1=d2b, op=OP.mult)
                nc.vector.tensor_tensor(out=h3, in0=h3, in1=rs3, op=OP.add)
                bb = bt[L][:].unsqueeze(1).broadcast_to((128, W, D))
                nc.vector.tensor_tensor(out=h3, in0=h3, in1=bb, op=OP.add)
                nc.scalar.activation(rs_sb[:], h_sb[:], act)
                # pooling (h lives in rs_sb/rs3 now)
                pps = ps_pool.tile([128, GW * D], F32, name="pool_ps",
                                   space="PSUM")
                for w in range(W):
                    for gw in range(GW):
                        nc.tensor.matmul(
                            pps[:, gw * D:(gw + 1) * D],
                            lhsT=eqp_sb[:, w * 256 + gw * 128:
                                        w * 256 + gw * 128 + 128],
                            rhs=rs3[:, w, :],
                            start=(w == 0 and gw == 0),
                            stop=(w == W - 1 and gw == GW - 1))
                pk = stage_p.tile([128, GW * D], F32, name="pk")
                nc.scalar.copy(pk[:], pps[:])
                nc.sync.dma_start(
                    pool_out[L][:].rearrange("(g r) e -> r g e", g=GW),
                    pk[:].rearrange("p (g e) -> p g e", g=GW))
                if L == 0:
                    nc.sync.dma_start(h0_v, rs_sb[:])
                    hT_sb = lhsT_p.tile([D, NPC], BF16, name="hT_sb")
                    nc.sync.dma_start_transpose(hT_sb[:], h0_d[:])

            # ---- program ----
            t_phase(0, xt_sb[:])
            stream(0)
            post(0)
            t_phase(1, hT_sb[:])
            stream(1)
            post(1)

    nc.finalize()
    return nc


_NC_CACHE = {}


def get_nc(cfg, Cs, act=AF.Gelu):
    key = (cfg.N, cfg.E, cfg.G, cfg.K, Cs, act)
    if key not in _NC_CACHE:
        _NC_CACHE[key] = build_nc(cfg, Cs, act=act)
    return _NC_CACHE[key]


def make_in_maps(cfg, data, W0, b0, W1, b1):
    D = cfg.D
    w0 = np.asarray(W0, np.float32).astype(NPBF)
    w1 = np.asarray(W1, np.float32).astype(NPBF)
    b0b = np.ascontiguousarray(np.broadcast_to(
        np.asarray(b0, np.float32).astype(NPBF), (128, D)))
    b1b = np.ascontiguousarray(np.broadcast_to(
        np.asarray(b1, np.float32).astype(NPBF), (128, D)))
    maps = []
    for k in range(cfg.K):
        m = {name: arr[k] for name, arr in data.items()}
        m.update({"w0": w0, "w1": w1, "b0b": b0b, "b1b": b1b})
        maps.append(m)
    return maps


def postprocess(cfg, results, counts):
    outs = []
    denom = np.maximum(counts, 1.0).astype(np.float32)
    for L in (0, 1):
        tot = np.zeros((cfg.GW * 128, cfg.D), dtype=np.float32)
        for k in range(cfg.K):
            tot += results[k][f"pool{L}"]
        outs.append((tot[:cfg.G] / denom[:, None]).astype(np.float32))
    return tuple(outs)


def kernel(x, edge_index, edge_weight, batch, W0, b0, W1, b1):
    cfg = FULL
    data, counts, Cs = prep_host(cfg, x, edge_index, edge_weight, batch)
    nc = get_nc(cfg, Cs)
    in_maps = make_in_maps(cfg, data, W0, b0, W1, b1)
    res = run_bass_kernel_spmd(nc, in_maps, list(range(cfg.K)))
    return postprocess(cfg, res.results, counts)


# revision 30
# speedup vs baseline: 3.1695x; 1.1218x over previous
"""2-layer GCN block (gcn_norm + 2x GCNConv/gelu + global mean pool) on
8 Trainium2 NeuronCores via Bass/Tile, SPMD, src-partitioned.

Design (v10):
  - Nodes renumbered and partitioned by OWNER core (12544 padded rows/core,
    98 windows of 128, 49 window-PAIRS/core, 392 global pairs). The GCN
    normalization (deg, dis, per-edge norm ewn = dis[s]*ew*dis[d], and the
    self-loop coefficient dis^2) is precomputed on the host like the
    per-graph node counts.
  - Each layer, each core computes t = h_own @ W for its OWN rows only
    (bf16), writes the 12544x(64+64pad) bf16 table to DRAM (256B rows).
  - Edges are processed by their SRC owner. One dma_gather per ~48-chunk
    batch pulls the per-edge src rows (128 rows/chunk) from the local
    table. Per chunk, TWO matmuls with a host-streamed fp8 one-hot matrix
    (dst-row-in-pair one-hot, scaled by ewn: the gather's edge scaling and
    the scatter indicator fused into static data) accumulate the chunk's
    messages into a [128, 2x64] PSUM tile per dst window-pair.
  - Completed pairs are copied PSUM->SBUF (bf16, DVE/Act alternating) and
    written to a [K*12544, 64] bf16 partial buffer; ONE ReduceScatter per
    layer (output only 1.6MB -> ~55us vs 284us for an AllGather) gives
    each core the full aggregation for its own rows.
  - Post: h = gelu(agg + dis2*t_own + b) batched over all windows; global
    mean pooling via host-streamed fp8 graph-indicator matmuls into PSUM;
    host divides by counts and sums the 8 per-core partials.
  - Layer 1 lhsT comes from ONE dma_start_transpose of h0 (no PE
    transposes).
"""
import numpy as np
import ml_dtypes

import concourse.bacc as bacc
import concourse.bass as bass
import concourse.mybir as mybir
import concourse.tile as tile
from concourse.bass_utils import run_bass_kernel_spmd
from concourse.library_config import mlp

F32 = mybir.dt.float32
BF16 = mybir.dt.bfloat16
FP8 = mybir.dt.float8e4
I16 = mybir.dt.int16
AF = mybir.ActivationFunctionType
OP = mybir.AluOpType

NPF8 = ml_dtypes.float8_e4m3fn
NPBF = ml_dtypes.bfloat16


class Cfg:
    def __init__(self, N=100000, E=1200000, D=64, G=256, K=8):
        self.N, self.E, self.D, self.G, self.K = N, E, D, G, K
        # 100 windows/core: mean edges per (core, window-pair) bucket is
        # E/K/400 = 375, comfortably under the 384 = 3*128 chunk boundary,
        # so the host balancer can hold nearly every bucket to 3 chunks.
        self.W = 100                  # windows per core
        self.NPC = self.W * 128       # padded rows per core (12800)
        self.PPC = self.W // 2        # pairs per core (50)
        self.NP = self.K * self.PPC   # global pairs (400)
        self.GW = -(-G // 128)        # graph-id windows (2)
        self.BCH = 24                 # chunks per gather/eqa batch
        self.SGRP = 10                # pairs per partial-write group


FULL = Cfg()


def _assign_pairs(vecs, npairs, cap):
    """Greedy multi-dim balance: assign each node (row of vecs [n, K]) to a
    pair, minimizing the resulting max per-src-core in-edge count, capacity
    `cap` nodes per pair. Returns pair index per node."""
    n, kk = vecs.shape
    order = np.argsort(-vecs.sum(1), kind="stable")
    S = np.zeros((npairs, kk), dtype=np.int64)
    cnt = np.zeros(npairs, dtype=np.int64)
    out = np.empty(n, dtype=np.int64)
    for i in order:
        v = vecs[i]
        score = (S + v).max(axis=1).astype(np.float64)
        score[cnt >= cap] = np.inf
        # tie-break on emptiest bin to keep counts even
        b = np.argmin(score + cnt * 1e-6)
        out[i] = b
        S[b] += v
        cnt[b] += 1
    return out


def prep_host(cfg, x, edge_index, edge_weight, batch):
    N, E, D, K, W = cfg.N, cfg.E, cfg.D, cfg.K, cfg.W
    NPC, PPC, NP = cfg.NPC, cfg.PPC, cfg.NP
    src = np.asarray(edge_index[0], dtype=np.int64)
    dst = np.asarray(edge_index[1], dtype=np.int64)
    ew = np.asarray(edge_weight, dtype=np.float64)
    batch = np.asarray(batch, dtype=np.int64)
    x = np.asarray(x, dtype=np.float32)

    # ---- gcn_norm on host (graph preprocessing, like the pool counts) ----
    deg = np.bincount(dst, weights=ew, minlength=N) + 1.0
    dis = 1.0 / np.sqrt(deg)
    ewn = (dis[src] * ew * dis[dst]).astype(np.float32)
    dis2 = (dis * dis).astype(np.float32)

    # ---- node -> (core, local row) numbering ----
    outdeg = np.bincount(src, minlength=N)
    order = np.argsort(-outdeg, kind="stable")
    ranks = np.arange(N)
    stratum, posin = ranks // K, ranks % K
    core_rank = np.where(stratum % 2 == 0, posin, K - 1 - posin)
    core_of = np.empty(N, dtype=np.int64)
    core_of[order] = core_rank

    # in-edge count of each node split by src core
    src_core = core_of[src]
    vq = np.bincount(dst * K + src_core, minlength=N * K).reshape(N, K)

    dloc = np.empty(N, dtype=np.int64)     # local row within the core
    for c in range(K):
        nodes = np.nonzero(core_of == c)[0]
        pair = _assign_pairs(vq[nodes], PPC, 256)
        posin_pair = np.zeros(len(nodes), dtype=np.int64)
        cnts = np.zeros(PPC, dtype=np.int64)
        for ii, p in enumerate(pair):
            posin_pair[ii] = cnts[p]
            cnts[p] += 1
        dloc[nodes] = pair * 256 + posin_pair

    grow = core_of * NPC + dloc            # node -> global padded row
    row_node = np.full(K * NPC, -1, dtype=np.int64)
    row_node[grow] = np.arange(N)

    # ---- edge slot schedule (shared across cores) ----
    e_core = src_core                                   # processing core
    e_pair = (core_of[dst] * PPC) + (dloc[dst] >> 8)    # global dst pair
    cntkp = np.bincount(e_core * NP + e_pair,
                        minlength=K * NP).reshape(K, NP)
    Cs = np.maximum(1, (cntkp.max(axis=0) + 127) // 128)        # [NP]
    off = np.zeros(NP + 1, dtype=np.int64)
    np.cumsum(Cs, out=off[1:])
    CT = int(off[-1])
    SLOTS = CT * 128

    bucket = e_core * NP + e_pair
    eorder = np.argsort(bucket, kind="stable")
    starts = np.zeros(K * NP, dtype=np.int64)
    np.cumsum(cntkp.ravel()[:-1], out=starts[1:])
    pos = np.arange(E) - starts[bucket[eorder]]
    slot = off[e_pair[eorder]] * 128 + pos              # slot within core
    es, ed, ewn_s = src[eorder], dst[eorder], ewn[eorder]
    ecore_s = e_core[eorder]

    # gather table row of src: r = p*W + w  (partition-major)
    sw, sp = dloc[es] >> 7, dloc[es] & 127
    tabrow = sp * W + sw
    # dst row-in-pair (0..255)
    jrow = dloc[ed] & 255

    # per-core streams; one-hot eqa holds EXACT 1.0 in fp8, the edge norm
    # rides in a separate f32 sidecar applied on the DVE
    idxw = np.zeros((K, 128, SLOTS // 16), dtype=np.int16)
    eqa = np.zeros((K, 128, CT * 256), dtype=NPF8)
    ewn_pm = np.zeros((K, 128, CT), dtype=np.float32)
    for c in range(K):
        m = ecore_s == c
        s_c, tr_c, j_c, wv_c = slot[m], tabrow[m], jrow[m], ewn_s[m]
        idxf = np.zeros(SLOTS, dtype=np.int16)
        idxf[s_c] = tr_c.astype(np.int16)
        iw = idxf.reshape(-1, 16).T                     # [16, SLOTS/16]
        idxw[c] = np.tile(iw, (8, 1))
        chunk = s_c >> 7
        erow = s_c & 127
        flat = erow * (CT * 256) + chunk * 256 + j_c
        ef = eqa[c].reshape(-1)
        ef[flat] = np.float32(1.0).astype(NPF8)
        ewn_pm[c][erow, chunk] = wv_c

    # ---- per-core node-indexed tensors ----
    real = row_node >= 0
    nid = np.maximum(row_node, 0)
    xw = np.where(real[:, None], x[nid], 0.0).astype(NPBF)   # [K*NPC, D]
    # x_t: [K, 64, NPC] columns in w-major local order (col = dloc)
    x_t = xw.reshape(K, NPC, D).transpose(0, 2, 1).copy()

    d2 = np.where(real, dis2[nid], 0.0).astype(np.float32)
    # dis2_pm [K, 128, W]: [p, w] = dis2 of dloc w*128+p
    dis2_pm = d2.reshape(K, W, 128).transpose(0, 2, 1).copy()

    bat = np.where(real, batch[nid], -1)
    eqp = np.zeros((K, 128, W * 256), dtype=NPF8)
    bkw = bat.reshape(K, W, 128)
    for c in range(K):
        p_i, w_i = np.meshgrid(np.arange(128), np.arange(W), indexing="ij")
        g = bkw[c].T                                    # [128, W]
        valid = g >= 0
        flat = (p_i * (W * 256) + w_i * 256 + g)[valid]
        ef = eqp[c].reshape(-1)
        ef[flat] = np.float32(1.0).astype(NPF8)

    counts = np.bincount(batch, minlength=cfg.G).astype(np.float32)
    data = {"x_t": x_t, "idxw": idxw, "eqa": eqa, "ewn": ewn_pm,
            "eqp": eqp, "dis2": dis2_pm}
    return data, counts, tuple(int(v) for v in Cs)


def build_nc(cfg, Cs, debug=False, act=AF.Gelu):
    K, W, NPC, D, GW = cfg.K, cfg.W, cfg.NPC, cfg.D, cfg.GW
    NP, PPC, BCH, SGRP = cfg.NP, cfg.PPC, cfg.BCH, cfg.SGRP
    off = [0]
    for c in Cs:
        off.append(off[-1] + c)
    CT = off[-1]
    # chunk -> pair, first/last flags
    cpair = np.empty(CT, dtype=np.int64)
    cfirst = np.zeros(CT, dtype=bool)
    clast = np.zeros(CT, dtype=bool)
    for p in range(NP):
        cpair[off[p]:off[p + 1]] = p
        cfirst[off[p]] = True
        clast[off[p + 1] - 1] = True
    batches = [(lo, min(lo + BCH, CT)) for lo in range(0, CT, BCH)]

    nc = bacc.Bacc("TRN2", target_bir_lowering=False, debug=debug)

    xt_d = nc.dram_tensor("x_t", [D, NPC], BF16, kind="ExternalInput")
    idx_d = nc.dram_tensor("idxw", [128, CT * 8], I16, kind="ExternalInput")
    eqa_d = nc.dram_tensor("eqa", [128, CT * 256], FP8, kind="ExternalInput")
    ewn_d = nc.dram_tensor("ewn", [128, CT], F32, kind="ExternalInput")
    eqp_d = nc.dram_tensor("eqp", [128, W * 256], FP8, kind="ExternalInput")
    dis2_d = nc.dram_tensor("dis2", [128, W], F32, kind="ExternalInput")
    w_d = [nc.dram_tensor(f"w{L}", [D, D], BF16, kind="ExternalInput")
           for L in (0, 1)]
    b_d = [nc.dram_tensor(f"b{L}b", [128, D], BF16, kind="ExternalInput")
           for L in (0, 1)]
    pool_out = [nc.dram_tensor(f"pool{L}", [GW * 128, D], F32,
                               kind="ExternalOutput") for L in (0, 1)]

    rg = [list(range(K))]

    with tile.TileContext(nc) as tc:
        with tc.tile_pool(name="const", bufs=1) as cpool, \
             tc.tile_pool(name="state", bufs=1) as spool, \
             tc.tile_pool(name="lhsT_p", bufs=1) as lhsT_p, \
             tc.tile_pool(name="dram", bufs=1, space="DRAM") as dpool, \
             tc.tile_pool(name="gath_p", bufs=2) as gath_p, \
             tc.tile_pool(name="eqa_p", bufs=2) as eqa_p, \
             tc.tile_pool(name="stage_p", bufs=3) as stage_p, \
             tc.tile_pool(name="ps_t", bufs=2, space="PSUM") as ps_t, \
             tc.tile_pool(name="ps_pair", bufs=3, space="PSUM") as ps_pair, \
             tc.tile_pool(name="ps_pool", bufs=2, space="PSUM") as ps_pool:

            nc.gpsimd.load_library(mlp)

            # consts
            wt = []
            for L in (0, 1):
                t = cpool.tile([D, D], BF16, name=f"wt{L}")
                nc.sync.dma_start(t[:], w_d[L][:])
                wt.append(t)
            bt = []
            for L in (0, 1):
                t = cpool.tile([128, D], BF16, name=f"bt{L}")
                nc.sync.dma_start(t[:], b_d[L][:])
                bt.append(t)
            dis2_sb = cpool.tile([128, W], F32, name="dis2_sb")
            nc.sync.dma_start(dis2_sb[:], dis2_d[:])
            eqp_sb = cpool.tile([128, W * 256], FP8, name="eqp_sb")
            nc.sync.dma_start(eqp_sb[:], eqp_d[:])
            idx_sb = cpool.tile([128, CT * 8], I16, name="idx_sb")
            nc.sync.dma_start(idx_sb[:], idx_d[:])
            ewn_sb = cpool.tile([128, CT], F32, name="ewn_sb")
            nc.sync.dma_start(ewn_sb[:], ewn_d[:])

            t_sb = spool.tile([128, W * 128], BF16, name="t_sb")
            nc.vector.memset(t_sb[:], 0)      # pad halves stay zero
            h_sb = spool.tile([128, W * D], BF16, name="h_sb")   # scratch
            rs_sb = spool.tile([128, W * D], BF16, name="rs_sb")  # rs, then h

            table_d = dpool.tile([NPC, 128], BF16, name="table")
            parts_d = dpool.tile([K * NPC, D], BF16, name="parts")
            rs_out_d = dpool.tile([NPC, D], BF16, name="rs_out")
            h0_d = dpool.tile([NPC, D], BF16, name="h0")

            xt_sb = lhsT_p.tile([D, NPC], BF16, name="xt_sb")
            nc.sync.dma_start(xt_sb[:], xt_d[:])

            # partials DRAM view: [k][p][q=(pair_local*2+w01)][64]
            parts_v = parts_d[:].rearrange("(k p q) e -> k p (q e)",
                                           k=K, p=128)
            # rs_out rows r=p*W+w -> per-partition contiguous
            rs_v = rs_out_d[:].rearrange("(p r) e -> p (r e)", p=128)
            # h0 rows d = w*128+p
            h0_v = h0_d[:].rearrange("(w p) e -> p w e", p=128)
            # table rows r = p*W+w
            tab_v = table_d[:].rearrange("(p r) e -> p (r e)", p=128)

            t3 = t_sb[:].rearrange("p (w e) -> p w e", e=128)
            h3 = h_sb[:].rearrange("p (w e) -> p w e", e=D)
            rs3 = rs_sb[:].rearrange("p (w e) -> p w e", e=D)

            hT_sb = None

            def t_phase(L, lhsT):
                # t = h @ W  (8 windows per PSUM bank)
                for wb in range(0, W, 8):
                    nwin = min(8, W - wb)
                    pt = ps_t.tile([128, nwin * D], F32, name="pt",
                                   space="PSUM")
                    for i in range(nwin):
                        w = wb + i
                        # one zero-region: first mm starts, last stops
                        nc.tensor.matmul(
                            pt[:, i * D:(i + 1) * D],
                            lhsT=lhsT[:, w * 128:(w + 1) * 128],
                            rhs=wt[L][:], start=(i == 0),
                            stop=(i == nwin - 1))
                    dst = t3[:, wb:wb + nwin, 0:D]
                    src = pt[:].rearrange("p (w e) -> p w e", e=D)
                    eng = nc.vector if (wb // 8) % 2 == 0 else nc.scalar
                    if eng is nc.vector:
                        eng.tensor_copy(dst, src)
                    else:
                        eng.copy(dst, src)
                nc.sync.dma_start(tab_v, t_sb[:])

            def stream(L):
                pair_ps = {}
                for (lo, hi) in batches:
                    nch = hi - lo
                    gath = gath_p.tile([128, BCH * 128], BF16, name="gath")
                    g3 = gath[:].rearrange("p (c e) -> p c e", e=128)
                    nc.gpsimd.dma_gather(
                        g3[:, 0:nch, :],
                        table_d[:], idx_sb[:, lo * 8:hi * 8],
                        nch * 128, nch * 128, 128, single_packet=False)
                    # scale the gathered rows by the f32 edge norm in place
                    wb_ = ewn_sb[:, lo:hi].unsqueeze(2).broadcast_to(
                        (128, nch, D))
                    nc.vector.tensor_tensor(out=g3[:, 0:nch, 0:D],
                                            in0=g3[:, 0:nch, 0:D],
                                            in1=wb_, op=OP.mult)
                    eqa_t = eqa_p.tile([128, BCH * 256], FP8, name="eqa_t")
                    nc.sync.dma_start(eqa_t[:, 0:nch * 256],
                                      eqa_d[:, lo * 256:hi * 256])
                    for c in range(lo, hi):
                        pr = int(cpair[c])
                        if cfirst[c]:
                            pair_ps[pr] = ps_pair.tile(
                                [128, 2 * D], F32, name="pp",
                                space="PSUM")
                        pp = pair_ps[pr]
                        cb = (c - lo) * 256
                        rhs = gath[:, (c - lo) * 128:(c - lo) * 128 + D]
                        # both windows share one psum zero-region: only the
                        # pair's very first mm starts it, the last stops it
                        nc.tensor.matmul(pp[:, 0:D],
                                         lhsT=eqa_t[:, cb:cb + 128],
                                         rhs=rhs, start=bool(cfirst[c]),
                                         stop=False)
                        nc.tensor.matmul(pp[:, D:2 * D],
                                         lhsT=eqa_t[:, cb + 128:cb + 256],
                                         rhs=rhs, start=False,
                                         stop=bool(clast[c]))
                        if clast[c]:
                            g = pr // SGRP
                            gslot = pr % SGRP
                            if gslot == 0:
                                stage_t = stage_p.tile(
                                    [128, SGRP * 2 * D], BF16, name="stage")
                                pair_ps["stage"] = stage_t
                            stage_t = pair_ps["stage"]
                            dstp = stage_t[:, gslot * 2 * D:(gslot + 1) * 2 * D]
                            if pr % 2 == 0:
                                nc.vector.tensor_copy(dstp, pp[:])
                            else:
                                nc.scalar.copy(dstp, pp[:])
                            del pair_ps[pr]
                            if gslot == SGRP - 1:
                                kd = pr // PPC
                                pl0 = (g % (PPC // SGRP)) * SGRP
                                nc.sync.dma_start(
                                    parts_v[kd][:, pl0 * 2 * D:
                                                (pl0 + SGRP) * 2 * D],
                                    stage_t[:])

            def post(L):
                nonlocal hT_sb
                nc.gpsimd.collective_compute(
                    "ReduceScatter", OP.add,
                    ins=[parts_d[:]], outs=[rs_out_d[:]], replica_groups=rg)
                nc.sync.dma_start(rs_sb[:], rs_v)
                # h = gelu(rs + dis2 * t + b); h_sb is scratch, the final
                # activations land in rs_sb (rs no longer needed then).
                d2b = dis2_sb[:].unsqueeze(2).broadcast_to((128, W, D))
                nc.vector.tensor_tensor(out=h3, in0=t3[:, :, 0:D],
                                        in1=d2b, op=OP.mult)
                nc.vector.tensor_tensor(out=h3, in0=h3, in1=rs3, op=OP.add)
                bb = bt[L][:].unsqueeze(1).broadcast_to((128, W, D))
                nc.vector.tensor_tensor(out=h3, in0=h3, in1=bb, op=OP.add)
                nc.scalar.activation(rs_sb[:], h_sb[:], act)
                # pooling (h lives in rs_sb/rs3 now)
                pps = ps_pool.tile([128, GW * D], F32, name="pool_ps",
                                   space="PSUM")
                for w in range(W):
                    for gw in range(GW):
                        nc.tensor.matmul(
                            pps[:, gw * D:(gw + 1) * D],
                            lhsT=eqp_sb[:, w * 256 + gw * 128:
                                        w * 256 + gw * 128 + 128],
                            rhs=rs3[:, w, :],
                            start=(w == 0 and gw == 0),
                            stop=(w == W - 1 and gw == GW - 1))
                pk = stage_p.tile([128, GW * D], F32, name="pk")
                nc.scalar.copy(pk[:], pps[:])
                nc.sync.dma_start(
                    pool_out[L][:].rearrange("(g r) e -> r g e", g=GW),
                    pk[:].rearrange("p (g e) -> p g e", g=GW))
                if L == 0:
                    nc.sync.dma_start(h0_v, rs_sb[:])
                    hT_sb = lhsT_p.tile([D, NPC], BF16, name="hT_sb")
                    nc.sync.dma_start_transpose(hT_sb[:], h0_d[:])

            # ---- program ----
            t_phase(0, xt_sb[:])
            stream(0)
            post(0)
            t_phase(1, hT_sb[:])
            stream(1)
            post(1)

    nc.finalize()
    return nc


_NC_CACHE = {}


def get_nc(cfg, Cs, act=AF.Gelu):
    key = (cfg.N, cfg.E, cfg.G, cfg.K, Cs, act)
    if key not in _NC_CACHE:
        _NC_CACHE[key] = build_nc(cfg, Cs, act=act)
    return _NC_CACHE[key]


def make_in_maps(cfg, data, W0, b0, W1, b1):
    D = cfg.D
    w0 = np.asarray(W0, np.float32).astype(NPBF)
    w1 = np.asarray(W1, np.float32).astype(NPBF)
    b0b = np.ascontiguousarray(np.broadcast_to(
        np.asarray(b0, np.float32).astype(NPBF), (128, D)))
    b1b = np.ascontiguousarray(np.broadcast_to(
        np.asarray(b1, np.float32).astype(NPBF), (128, D)))
    maps = []
    for k in range(cfg.K):
        m = {name: arr[k] for name, arr in data.items()}
        m.update({"w0": w0, "w1": w1, "b0b": b0b, "b1b": b1b})
        maps.append(m)
    return maps


def postprocess(cfg, results, counts):
    outs = []
    denom = np.maximum(counts, 1.0).astype(np.float32)
    for L in (0, 1):
        tot = np.zeros((cfg.GW * 128, cfg.D), dtype=np.float32)
        for k in range(cfg.K):
            tot += results[k][f"pool{L}"]
        outs.append((tot[:cfg.G] / denom[:, None]).astype(np.float32))
    return tuple(outs)


def kernel(x, edge_index, edge_weight, batch, W0, b0, W1, b1):
    cfg = FULL
    data, counts, Cs = prep_host(cfg, x, edge_index, edge_weight, batch)
    nc = get_nc(cfg, Cs)
    in_maps = make_in_maps(cfg, data, W0, b0, W1, b1)
    res = run_bass_kernel_spmd(nc, in_maps, list(range(cfg.K)))
    return postprocess(cfg, res.results, counts)


# revision 39
# speedup vs baseline: 3.3375x; 1.0530x over previous
"""2-layer GCN block (gcn_norm + 2x GCNConv/gelu + global mean pool) on
8 Trainium2 NeuronCores via Bass/Tile, SPMD, src-partitioned.

Design (v10):
  - Nodes renumbered and partitioned by OWNER core (12544 padded rows/core,
    98 windows of 128, 49 window-PAIRS/core, 392 global pairs). The GCN
    normalization (deg, dis, per-edge norm ewn = dis[s]*ew*dis[d], and the
    self-loop coefficient dis^2) is precomputed on the host like the
    per-graph node counts.
  - Each layer, each core computes t = h_own @ W for its OWN rows only
    (bf16), writes the 12544x(64+64pad) bf16 table to DRAM (256B rows).
  - Edges are processed by their SRC owner. One dma_gather per ~48-chunk
    batch pulls the per-edge src rows (128 rows/chunk) from the local
    table. Per chunk, TWO matmuls with a host-streamed fp8 one-hot matrix
    (dst-row-in-pair one-hot, scaled by ewn: the gather's edge scaling and
    the scatter indicator fused into static data) accumulate the chunk's
    messages into a [128, 2x64] PSUM tile per dst window-pair.
  - Completed pairs are copied PSUM->SBUF (bf16, DVE/Act alternating) and
    written to a [K*12544, 64] bf16 partial buffer; ONE ReduceScatter per
    layer (output only 1.6MB -> ~55us vs 284us for an AllGather) gives
    each core the full aggregation for its own rows.
  - Post: h = gelu(agg + dis2*t_own + b) batched over all windows; global
    mean pooling via host-streamed fp8 graph-indicator matmuls into PSUM;
    host divides by counts and sums the 8 per-core partials.
  - Layer 1 lhsT comes from ONE dma_start_transpose of h0 (no PE
    transposes).
"""
import numpy as np
import ml_dtypes

import concourse.bacc as bacc
import concourse.bass as bass
import concourse.mybir as mybir
import concourse.tile as tile
from concourse.bass_utils import run_bass_kernel_spmd
from concourse.library_config import mlp

F32 = mybir.dt.float32
BF16 = mybir.dt.bfloat16
FP8 = mybir.dt.float8e4
I16 = mybir.dt.int16
AF = mybir.ActivationFunctionType
OP = mybir.AluOpType

NPF8 = ml_dtypes.float8_e4m3fn
NPBF = ml_dtypes.bfloat16


class Cfg:
    def __init__(self, N=100000, E=1200000, D=64, G=256, K=8):
        self.N, self.E, self.D, self.G, self.K = N, E, D, G, K
        # 100 windows/core: mean edges per (core, window-pair) bucket is
        # E/K/400 = 375, comfortably under the 384 = 3*128 chunk boundary,
        # so the host balancer can hold nearly every bucket to 3 chunks.
        self.W = 100                  # windows per core
        self.NPC = self.W * 128       # padded rows per core (12800)
        self.PPC = self.W // 2        # pairs per core (50)
        self.NP = self.K * self.PPC   # global pairs (400)
        self.GW = -(-G // 128)        # graph-id windows (2)
        self.BCH = 24                 # chunks per gather/eqa batch
        self.SGRP = 10                # pairs per partial-write group


FULL = Cfg()


def _assign_pairs(vecs, npairs, cap):
    """Greedy multi-dim balance: assign each node (row of vecs [n, K]) to a
    pair, minimizing the resulting max per-src-core in-edge count, capacity
    `cap` nodes per pair. Returns pair index per node."""
    n, kk = vecs.shape
    order = np.argsort(-vecs.sum(1), kind="stable")
    S = np.zeros((npairs, kk), dtype=np.int64)
    cnt = np.zeros(npairs, dtype=np.int64)
    out = np.empty(n, dtype=np.int64)
    for i in order:
        v = vecs[i]
        score = (S + v).max(axis=1).astype(np.float64)
        score[cnt >= cap] = np.inf
        # tie-break on emptiest bin to keep counts even
        b = np.argmin(score + cnt * 1e-6)
        out[i] = b
        S[b] += v
        cnt[b] += 1
    return out


def prep_host(cfg, x, edge_index, edge_weight, batch):
    N, E, D, K, W = cfg.N, cfg.E, cfg.D, cfg.K, cfg.W
    NPC, PPC, NP = cfg.NPC, cfg.PPC, cfg.NP
    src = np.asarray(edge_index[0], dtype=np.int64)
    dst = np.asarray(edge_index[1], dtype=np.int64)
    ew = np.asarray(edge_weight, dtype=np.float64)
    batch = np.asarray(batch, dtype=np.int64)
    x = np.asarray(x, dtype=np.float32)

    # ---- gcn_norm on host (graph preprocessing, like the pool counts) ----
    deg = np.bincount(dst, weights=ew, minlength=N) + 1.0
    dis = 1.0 / np.sqrt(deg)
    ewn = (dis[src] * ew * dis[dst]).astype(np.float32)
    dis2 = (dis * dis).astype(np.float32)

    # ---- node -> (core, local row) numbering ----
    outdeg = np.bincount(src, minlength=N)
    order = np.argsort(-outdeg, kind="stable")
    ranks = np.arange(N)
    stratum, posin = ranks // K, ranks % K
    core_rank = np.where(stratum % 2 == 0, posin, K - 1 - posin)
    core_of = np.empty(N, dtype=np.int64)
    core_of[order] = core_rank

    # in-edge count of each node split by src core
    src_core = core_of[src]
    vq = np.bincount(dst * K + src_core, minlength=N * K).reshape(N, K)

    dloc = np.empty(N, dtype=np.int64)     # local row within the core
    for c in range(K):
        nodes = np.nonzero(core_of == c)[0]
        pair = _assign_pairs(vq[nodes], PPC, 256)
        posin_pair = np.zeros(len(nodes), dtype=np.int64)
        cnts = np.zeros(PPC, dtype=np.int64)
        for ii, p in enumerate(pair):
            posin_pair[ii] = cnts[p]
            cnts[p] += 1
        dloc[nodes] = pair * 256 + posin_pair

    grow = core_of * NPC + dloc            # node -> global padded row
    row_node = np.full(K * NPC, -1, dtype=np.int64)
    row_node[grow] = np.arange(N)

    # ---- edge slot schedule (shared across cores) ----
    e_core = src_core                                   # processing core
    e_pair = (core_of[dst] * PPC) + (dloc[dst] >> 8)    # global dst pair
    cntkp = np.bincount(e_core * NP + e_pair,
                        minlength=K * NP).reshape(K, NP)
    Cs = np.maximum(1, (cntkp.max(axis=0) + 127) // 128)        # [NP]
    off = np.zeros(NP + 1, dtype=np.int64)
    np.cumsum(Cs, out=off[1:])
    CT = int(off[-1])
    SLOTS = CT * 128

    bucket = e_core * NP + e_pair
    eorder = np.argsort(bucket, kind="stable")
    starts = np.zeros(K * NP, dtype=np.int64)
    np.cumsum(cntkp.ravel()[:-1], out=starts[1:])
    pos = np.arange(E) - starts[bucket[eorder]]
    slot = off[e_pair[eorder]] * 128 + pos              # slot within core
    es, ed, ewn_s = src[eorder], dst[eorder], ewn[eorder]
    ecore_s = e_core[eorder]

    # gather table row of src: r = p*W + w  (partition-major)
    sw, sp = dloc[es] >> 7, dloc[es] & 127
    tabrow = sp * W + sw
    # dst row-in-pair (0..255)
    jrow = dloc[ed] & 255

    # per-core streams; one-hot eqa holds EXACT 1.0 in fp8, the edge norm
    # rides in a separate f32 sidecar applied on the DVE
    idxw = np.zeros((K, 128, SLOTS // 16), dtype=np.int16)
    eqa = np.zeros((K, 128, CT * 256), dtype=NPF8)
    ewn_pm = np.zeros((K, 128, CT), dtype=np.float32)
    for c in range(K):
        m = ecore_s == c
        s_c, tr_c, j_c, wv_c = slot[m], tabrow[m], jrow[m], ewn_s[m]
        idxf = np.zeros(SLOTS, dtype=np.int16)
        idxf[s_c] = tr_c.astype(np.int16)
        iw = idxf.reshape(-1, 16).T                     # [16, SLOTS/16]
        idxw[c] = np.tile(iw, (8, 1))
        chunk = s_c >> 7
        erow = s_c & 127
        flat = erow * (CT * 256) + chunk * 256 + j_c
        ef = eqa[c].reshape(-1)
        ef[flat] = np.float32(1.0).astype(NPF8)
        ewn_pm[c][erow, chunk] = wv_c

    # ---- per-core node-indexed tensors ----
    real = row_node >= 0
    nid = np.maximum(row_node, 0)
    xw = np.where(real[:, None], x[nid], 0.0).astype(NPBF)   # [K*NPC, D]
    # x_t: [K, 64, NPC] columns in w-major local order (col = dloc)
    x_t = xw.reshape(K, NPC, D).transpose(0, 2, 1).copy()

    d2 = np.where(real, dis2[nid], 0.0).astype(np.float32)
    # dis2_pm [K, 128, W]: [p, w] = dis2 of dloc w*128+p
    dis2_pm = d2.reshape(K, W, 128).transpose(0, 2, 1).copy()

    bat = np.where(real, batch[nid], -1)
    eqp = np.zeros((K, 128, W * 256), dtype=NPF8)
    bkw = bat.reshape(K, W, 128)
    for c in range(K):
        p_i, w_i = np.meshgrid(np.arange(128), np.arange(W), indexing="ij")
        g = bkw[c].T                                    # [128, W]
        valid = g >= 0
        flat = (p_i * (W * 256) + w_i * 256 + g)[valid]
        ef = eqp[c].reshape(-1)
        ef[flat] = np.float32(1.0).astype(NPF8)

    counts = np.bincount(batch, minlength=cfg.G).astype(np.float32)
    data = {"x_t": x_t, "idxw": idxw, "eqa": eqa, "ewn": ewn_pm,
            "eqp": eqp, "dis2": dis2_pm}
    return data, counts, tuple(int(v) for v in Cs)


def build_nc(cfg, Cs, debug=False, act=AF.Gelu):
    K, W, NPC, D, GW = cfg.K, cfg.W, cfg.NPC, cfg.D, cfg.GW
    NP, PPC, BCH, SGRP = cfg.NP, cfg.PPC, cfg.BCH, cfg.SGRP
    off = [0]
    for c in Cs:
        off.append(off[-1] + c)
    CT = off[-1]
    # chunk -> pair, first/last flags
    cpair = np.empty(CT, dtype=np.int64)
    cfirst = np.zeros(CT, dtype=bool)
    clast = np.zeros(CT, dtype=bool)
    for p in range(NP):
        cpair[off[p]:off[p + 1]] = p
        cfirst[off[p]] = True
        clast[off[p + 1] - 1] = True
    batches = [(lo, min(lo + BCH, CT)) for lo in range(0, CT, BCH)]

    nc = bacc.Bacc("TRN2", target_bir_lowering=False, debug=debug)

    xt_d = nc.dram_tensor("x_t", [D, NPC], BF16, kind="ExternalInput")
    idx_d = nc.dram_tensor("idxw", [128, CT * 8], I16, kind="ExternalInput")
    eqa_d = nc.dram_tensor("eqa", [128, CT * 256], FP8, kind="ExternalInput")
    ewn_d = nc.dram_tensor("ewn", [128, CT], F32, kind="ExternalInput")
    eqp_d = nc.dram_tensor("eqp", [128, W * 256], FP8, kind="ExternalInput")
    dis2_d = nc.dram_tensor("dis2", [128, W], F32, kind="ExternalInput")
    w_d = [nc.dram_tensor(f"w{L}", [D, D], BF16, kind="ExternalInput")
           for L in (0, 1)]
    b_d = [nc.dram_tensor(f"b{L}b", [128, D], BF16, kind="ExternalInput")
           for L in (0, 1)]
    pool_out = [nc.dram_tensor(f"pool{L}", [GW * 128, D], F32,
                               kind="ExternalOutput") for L in (0, 1)]

    rg = [list(range(K))]

    with tile.TileContext(nc) as tc:
        with tc.tile_pool(name="const", bufs=1) as cpool, \
             tc.tile_pool(name="state", bufs=1) as spool, \
             tc.tile_pool(name="lhsT_p", bufs=1) as lhsT_p, \
             tc.tile_pool(name="dram", bufs=1, space="DRAM") as dpool, \
             tc.tile_pool(name="gath_p", bufs=2) as gath_p, \
             tc.tile_pool(name="eqa_p", bufs=6) as eqa_p, \
             tc.tile_pool(name="stage_p", bufs=3) as stage_p, \
             tc.tile_pool(name="ps_t", bufs=2, space="PSUM") as ps_t, \
             tc.tile_pool(name="ps_pair", bufs=4, space="PSUM") as ps_pair, \
             tc.tile_pool(name="ps_pool", bufs=2, space="PSUM") as ps_pool:

            nc.gpsimd.load_library(mlp)

            # consts
            wt = []
            for L in (0, 1):
                t = cpool.tile([D, D], BF16, name=f"wt{L}")
                nc.sync.dma_start(t[:], w_d[L][:])
                wt.append(t)
            bt = []
            for L in (0, 1):
                t = cpool.tile([128, D], BF16, name=f"bt{L}")
                nc.sync.dma_start(t[:], b_d[L][:])
                bt.append(t)
            dis2_sb = cpool.tile([128, W], F32, name="dis2_sb")
            nc.scalar.dma_start(dis2_sb[:], dis2_d[:])
            eqp_sb = cpool.tile([128, W * 256], FP8, name="eqp_sb")
            nc.scalar.dma_start(eqp_sb[:], eqp_d[:])
            idx_sb = cpool.tile([128, CT * 8], I16, name="idx_sb")
            nc.sync.dma_start(idx_sb[:], idx_d[:])
            ewn_sb = cpool.tile([128, CT], F32, name="ewn_sb")
            nc.sync.dma_start(ewn_sb[:], ewn_d[:])

            t_sb = spool.tile([128, W * 128], BF16, name="t_sb")
            nc.vector.memset(t_sb[:], 0)      # pad halves stay zero
            h_sb = spool.tile([128, W * D], BF16, name="h_sb")   # scratch
            rs_sb = spool.tile([128, W * D], BF16, name="rs_sb")  # rs, then h

            table_d = dpool.tile([NPC, 128], BF16, name="table")
            parts_d = dpool.tile([K * NPC, D], BF16, name="parts")
            rs_out_d = dpool.tile([NPC, D], BF16, name="rs_out")
            h0_d = dpool.tile([NPC, D], BF16, name="h0")

            xt_sb = lhsT_p.tile([D, NPC], BF16, name="xt_sb")
            nc.scalar.dma_start(xt_sb[:], xt_d[:])

            # partials DRAM view: [k][p][q=(pair_local*2+w01)][64]
            parts_v = parts_d[:].rearrange("(k p q) e -> k p (q e)",
                                           k=K, p=128)
            # rs_out rows r=p*W+w -> per-partition contiguous
            rs_v = rs_out_d[:].rearrange("(p r) e -> p (r e)", p=128)
            # h0 rows d = w*128+p
            h0_v = h0_d[:].rearrange("(w p) e -> p w e", p=128)
            # table rows r = p*W+w
            tab_v = table_d[:].rearrange("(p r) e -> p (r e)", p=128)

            t3 = t_sb[:].rearrange("p (w e) -> p w e", e=128)
            h3 = h_sb[:].rearrange("p (w e) -> p w e", e=D)
            rs3 = rs_sb[:].rearrange("p (w e) -> p w e", e=D)

            hT_sb = None

            def t_phase(L, lhsT):
                # t = h @ W  (8 windows per PSUM bank)
                for wb in range(0, W, 8):
                    nwin = min(8, W - wb)
                    pt = ps_t.tile([128, nwin * D], F32, name="pt",
                                   space="PSUM")
                    for i in range(nwin):
                        w = wb + i
                        # one zero-region: first mm starts, last stops
                        nc.tensor.matmul(
                            pt[:, i * D:(i + 1) * D],
                            lhsT=lhsT[:, w * 128:(w + 1) * 128],
                            rhs=wt[L][:], start=(i == 0),
                            stop=(i == nwin - 1))
                    dst = t3[:, wb:wb + nwin, 0:D]
                    src = pt[:].rearrange("p (w e) -> p w e", e=D)
                    eng = nc.vector if (wb // 8) % 2 == 0 else nc.scalar
                    if eng is nc.vector:
                        eng.tensor_copy(dst, src)
                    else:
                        eng.copy(dst, src)
                # scalar queue: keep SP free for stream prefetch run-ahead
                nc.scalar.dma_start(tab_v, t_sb[:])

            def stream(L):
                pair_ps = {}
                for (lo, hi) in batches:
                    nch = hi - lo
                    gath = gath_p.tile([128, BCH * 128], BF16, name="gath")
                    g3 = gath[:].rearrange("p (c e) -> p c e", e=128)
                    nc.gpsimd.dma_gather(
                        g3[:, 0:nch, :],
                        table_d[:], idx_sb[:, lo * 8:hi * 8],
                        nch * 128, nch * 128, 128, single_packet=False)
                    # scale the gathered rows by the f32 edge norm in place
                    wb_ = ewn_sb[:, lo:hi].unsqueeze(2).broadcast_to(
                        (128, nch, D))
                    nc.vector.tensor_tensor(out=g3[:, 0:nch, 0:D],
                                            in0=g3[:, 0:nch, 0:D],
                                            in1=wb_, op=OP.mult)
                    eqa_t = eqa_p.tile([128, BCH * 256], FP8, name="eqa_t")
                    nc.sync.dma_start(eqa_t[:, 0:nch * 256],
                                      eqa_d[:, lo * 256:hi * 256])
                    for c in range(lo, hi):
                        pr = int(cpair[c])
                        if cfirst[c]:
                            pair_ps[pr] = ps_pair.tile(
                                [128, 2 * D], F32, name="pp",
                                space="PSUM")
                        pp = pair_ps[pr]
                        cb = (c - lo) * 256
                        rhs = gath[:, (c - lo) * 128:(c - lo) * 128 + D]
                        # both windows share one psum zero-region: only the
                        # pair's very first mm starts it, the last stops it
                        nc.tensor.matmul(pp[:, 0:D],
                                         lhsT=eqa_t[:, cb:cb + 128],
                                         rhs=rhs, start=bool(cfirst[c]),
                                         stop=False)
                        nc.tensor.matmul(pp[:, D:2 * D],
                                         lhsT=eqa_t[:, cb + 128:cb + 256],
                                         rhs=rhs, start=False,
                                         stop=bool(clast[c]))
                        if clast[c]:
                            g = pr // SGRP
                            gslot = pr % SGRP
                            if gslot == 0:
                                stage_t = stage_p.tile(
                                    [128, SGRP * 2 * D], BF16, name="stage")
                                pair_ps["stage"] = stage_t
                            stage_t = pair_ps["stage"]
                            dstp = stage_t[:, gslot * 2 * D:(gslot + 1) * 2 * D]
                            if pr % 2 == 0:
                                nc.vector.tensor_copy(dstp, pp[:])
                            else:
                                nc.scalar.copy(dstp, pp[:])
                            del pair_ps[pr]
                            if gslot == SGRP - 1:
                                kd = pr // PPC
                                pl0 = (g % (PPC // SGRP)) * SGRP
                                nc.sync.dma_start(
                                    parts_v[kd][:, pl0 * 2 * D:
                                                (pl0 + SGRP) * 2 * D],
                                    stage_t[:])

            def post(L):
                nonlocal hT_sb
                nc.gpsimd.collective_compute(
                    "ReduceScatter", OP.add,
                    ins=[parts_d[:]], outs=[rs_out_d[:]], replica_groups=rg)
                # scalar queue: the rs load waits on the collective; on SP it
                # would head-of-line-block the next layer's eqa prefetch
                nc.scalar.dma_start(rs_sb[:], rs_v)
                # h = gelu(rs + dis2 * t + b); h_sb is scratch, the final
                # activations land in rs_sb (rs no longer needed then).
                d2b = dis2_sb[:].unsqueeze(2).broadcast_to((128, W, D))
                nc.vector.tensor_tensor(out=h3, in0=t3[:, :, 0:D],
                                        in1=d2b, op=OP.mult)
                nc.vector.tensor_tensor(out=h3, in0=h3, in1=rs3, op=OP.add)
                bb = bt[L][:].unsqueeze(1).broadcast_to((128, W, D))
                nc.vector.tensor_tensor(out=h3, in0=h3, in1=bb, op=OP.add)
                nc.scalar.activation(rs_sb[:], h_sb[:], act)
                # pooling (h lives in rs_sb/rs3 now)
                pps = ps_pool.tile([128, GW * D], F32, name="pool_ps",
                                   space="PSUM")
                for w in range(W):
                    for gw in range(GW):
                        nc.tensor.matmul(
                            pps[:, gw * D:(gw + 1) * D],
                            lhsT=eqp_sb[:, w * 256 + gw * 128:
                                        w * 256 + gw * 128 + 128],
                            rhs=rs3[:, w, :],
                            start=(w == 0 and gw == 0),
                            stop=(w == W - 1 and gw == GW - 1))
                pk = stage_p.tile([128, GW * D], F32, name="pk")
                nc.scalar.copy(pk[:], pps[:])
                nc.scalar.dma_start(
                    pool_out[L][:].rearrange("(g r) e -> r g e", g=GW),
                    pk[:].rearrange("p (g e) -> p g e", g=GW))
                if L == 0:
                    nc.scalar.dma_start(h0_v, rs_sb[:])
                    hT_sb = lhsT_p.tile([D, NPC], BF16, name="hT_sb")
                    nc.scalar.dma_start_transpose(hT_sb[:], h0_d[:])

            # ---- program ----
            t_phase(0, xt_sb[:])
            stream(0)
            post(0)
            t_phase(1, hT_sb[:])
            stream(1)
            post(1)

    nc.finalize()
    return nc


_NC_CACHE = {}


def get_nc(cfg, Cs, act=AF.Gelu):
    key = (cfg.N, cfg.E, cfg.G, cfg.K, Cs, act)
    if key not in _NC_CACHE:
        _NC_CACHE[key] = build_nc(cfg, Cs, act=act)
    return _NC_CACHE[key]


def make_in_maps(cfg, data, W0, b0, W1, b1):
    D = cfg.D
    w0 = np.asarray(W0, np.float32).astype(NPBF)
    w1 = np.asarray(W1, np.float32).astype(NPBF)
    b0b = np.ascontiguousarray(np.broadcast_to(
        np.asarray(b0, np.float32).astype(NPBF), (128, D)))
    b1b = np.ascontiguousarray(np.broadcast_to(
        np.asarray(b1, np.float32).astype(NPBF), (128, D)))
    maps = []
    for k in range(cfg.K):
        m = {name: arr[k] for name, arr in data.items()}
        m.update({"w0": w0, "w1": w1, "b0b": b0b, "b1b": b1b})
        maps.append(m)
    return maps


def postprocess(cfg, results, counts):
    outs = []
    denom = np.maximum(counts, 1.0).astype(np.float32)
    for L in (0, 1):
        tot = np.zeros((cfg.GW * 128, cfg.D), dtype=np.float32)
        for k in range(cfg.K):
            tot += results[k][f"pool{L}"]
        outs.append((tot[:cfg.G] / denom[:, None]).astype(np.float32))
    return tuple(outs)


def kernel(x, edge_index, edge_weight, batch, W0, b0, W1, b1):
    cfg = FULL
    data, counts, Cs = prep_host(cfg, x, edge_index, edge_weight, batch)
    nc = get_nc(cfg, Cs)
    in_maps = make_in_maps(cfg, data, W0, b0, W1, b1)
    res = run_bass_kernel_spmd(nc, in_maps, list(range(cfg.K)))
    return postprocess(cfg, res.results, counts)


# revision 42
# speedup vs baseline: 3.3655x; 1.0084x over previous
"""2-layer GCN block (gcn_norm + 2x GCNConv/gelu + global mean pool) on
8 Trainium2 NeuronCores via Bass/Tile, SPMD, src-partitioned.

Design (v10):
  - Nodes renumbered and partitioned by OWNER core (12544 padded rows/core,
    98 windows of 128, 49 window-PAIRS/core, 392 global pairs). The GCN
    normalization (deg, dis, per-edge norm ewn = dis[s]*ew*dis[d], and the
    self-loop coefficient dis^2) is precomputed on the host like the
    per-graph node counts.
  - Each layer, each core computes t = h_own @ W for its OWN rows only
    (bf16), writes the 12544x(64+64pad) bf16 table to DRAM (256B rows).
  - Edges are processed by their SRC owner. One dma_gather per ~48-chunk
    batch pulls the per-edge src rows (128 rows/chunk) from the local
    table. Per chunk, TWO matmuls with a host-streamed fp8 one-hot matrix
    (dst-row-in-pair one-hot, scaled by ewn: the gather's edge scaling and
    the scatter indicator fused into static data) accumulate the chunk's
    messages into a [128, 2x64] PSUM tile per dst window-pair.
  - Completed pairs are copied PSUM->SBUF (bf16, DVE/Act alternating) and
    written to a [K*12544, 64] bf16 partial buffer; ONE ReduceScatter per
    layer (output only 1.6MB -> ~55us vs 284us for an AllGather) gives
    each core the full aggregation for its own rows.
  - Post: h = gelu(agg + dis2*t_own + b) batched over all windows; global
    mean pooling via host-streamed fp8 graph-indicator matmuls into PSUM;
    host divides by counts and sums the 8 per-core partials.
  - Layer 1 lhsT comes from ONE dma_start_transpose of h0 (no PE
    transposes).
"""
import numpy as np
import ml_dtypes

import concourse.bacc as bacc
import concourse.bass as bass
import concourse.mybir as mybir
import concourse.tile as tile
from concourse.bass_utils import run_bass_kernel_spmd
from concourse.library_config import mlp

F32 = mybir.dt.float32
BF16 = mybir.dt.bfloat16
FP8 = mybir.dt.float8e4
I16 = mybir.dt.int16
AF = mybir.ActivationFunctionType
OP = mybir.AluOpType

NPF8 = ml_dtypes.float8_e4m3fn
NPBF = ml_dtypes.bfloat16


class Cfg:
    def __init__(self, N=100000, E=1200000, D=64, G=256, K=8):
        self.N, self.E, self.D, self.G, self.K = N, E, D, G, K
        # 100 windows/core: mean edges per (core, window-pair) bucket is
        # E/K/400 = 375, comfortably under the 384 = 3*128 chunk boundary,
        # so the host balancer can hold nearly every bucket to 3 chunks.
        self.W = 100                  # windows per core
        self.NPC = self.W * 128       # padded rows per core (12800)
        self.PPC = self.W // 2        # pairs per core (50)
        self.NP = self.K * self.PPC   # global pairs (400)
        self.GW = -(-G // 128)        # graph-id windows (2)
        self.BCH = 16                 # chunks per gather/eqa batch
        self.SGRP = 10                # pairs per partial-write group


FULL = Cfg()


def _assign_pairs(vecs, npairs, cap):
    """Greedy multi-dim balance: assign each node (row of vecs [n, K]) to a
    pair, minimizing the resulting max per-src-core in-edge count, capacity
    `cap` nodes per pair. Returns pair index per node."""
    n, kk = vecs.shape
    order = np.argsort(-vecs.sum(1), kind="stable")
    S = np.zeros((npairs, kk), dtype=np.int64)
    cnt = np.zeros(npairs, dtype=np.int64)
    out = np.empty(n, dtype=np.int64)
    for i in order:
        v = vecs[i]
        score = (S + v).max(axis=1).astype(np.float64)
        score[cnt >= cap] = np.inf
        # tie-break on emptiest bin to keep counts even
        b = np.argmin(score + cnt * 1e-6)
        out[i] = b
        S[b] += v
        cnt[b] += 1
    return out


def prep_host(cfg, x, edge_index, edge_weight, batch):
    N, E, D, K, W = cfg.N, cfg.E, cfg.D, cfg.K, cfg.W
    NPC, PPC, NP = cfg.NPC, cfg.PPC, cfg.NP
    src = np.asarray(edge_index[0], dtype=np.int64)
    dst = np.asarray(edge_index[1], dtype=np.int64)
    ew = np.asarray(edge_weight, dtype=np.float64)
    batch = np.asarray(batch, dtype=np.int64)
    x = np.asarray(x, dtype=np.float32)

    # ---- gcn_norm on host (graph preprocessing, like the pool counts) ----
    deg = np.bincount(dst, weights=ew, minlength=N) + 1.0
    dis = 1.0 / np.sqrt(deg)
    ewn = (dis[src] * ew * dis[dst]).astype(np.float32)
    dis2 = (dis * dis).astype(np.float32)

    # ---- node -> (core, local row) numbering ----
    outdeg = np.bincount(src, minlength=N)
    order = np.argsort(-outdeg, kind="stable")
    ranks = np.arange(N)
    stratum, posin = ranks // K, ranks % K
    core_rank = np.where(stratum % 2 == 0, posin, K - 1 - posin)
    core_of = np.empty(N, dtype=np.int64)
    core_of[order] = core_rank

    # in-edge count of each node split by src core
    src_core = core_of[src]
    vq = np.bincount(dst * K + src_core, minlength=N * K).reshape(N, K)

    dloc = np.empty(N, dtype=np.int64)     # local row within the core
    for c in range(K):
        nodes = np.nonzero(core_of == c)[0]
        pair = _assign_pairs(vq[nodes], PPC, 256)
        posin_pair = np.zeros(len(nodes), dtype=np.int64)
        cnts = np.zeros(PPC, dtype=np.int64)
        for ii, p in enumerate(pair):
            posin_pair[ii] = cnts[p]
            cnts[p] += 1
        dloc[nodes] = pair * 256 + posin_pair

    grow = core_of * NPC + dloc            # node -> global padded row
    row_node = np.full(K * NPC, -1, dtype=np.int64)
    row_node[grow] = np.arange(N)

    # ---- edge slot schedule (shared across cores) ----
    e_core = src_core                                   # processing core
    e_pair = (core_of[dst] * PPC) + (dloc[dst] >> 8)    # global dst pair
    cntkp = np.bincount(e_core * NP + e_pair,
                        minlength=K * NP).reshape(K, NP)
    Cs = np.maximum(1, (cntkp.max(axis=0) + 127) // 128)        # [NP]
    off = np.zeros(NP + 1, dtype=np.int64)
    np.cumsum(Cs, out=off[1:])
    CT = int(off[-1])
    SLOTS = CT * 128

    bucket = e_core * NP + e_pair
    eorder = np.argsort(bucket, kind="stable")
    starts = np.zeros(K * NP, dtype=np.int64)
    np.cumsum(cntkp.ravel()[:-1], out=starts[1:])
    pos = np.arange(E) - starts[bucket[eorder]]
    slot = off[e_pair[eorder]] * 128 + pos              # slot within core
    es, ed, ewn_s = src[eorder], dst[eorder], ewn[eorder]
    ecore_s = e_core[eorder]

    # gather table row of src: r = p*W + w  (partition-major)
    sw, sp = dloc[es] >> 7, dloc[es] & 127
    tabrow = sp * W + sw
    # dst row-in-pair (0..255)
    jrow = dloc[ed] & 255

    # per-core streams; one-hot eqa holds EXACT 1.0 in fp8, the edge norm
    # rides in a separate f32 sidecar applied on the DVE
    idxw = np.zeros((K, 128, SLOTS // 16), dtype=np.int16)
    eqa = np.zeros((K, 128, CT * 256), dtype=NPF8)
    ewn_pm = np.zeros((K, 128, CT), dtype=np.float32)
    for c in range(K):
        m = ecore_s == c
        s_c, tr_c, j_c, wv_c = slot[m], tabrow[m], jrow[m], ewn_s[m]
        idxf = np.zeros(SLOTS, dtype=np.int16)
        idxf[s_c] = tr_c.astype(np.int16)
        iw = idxf.reshape(-1, 16).T                     # [16, SLOTS/16]
        idxw[c] = np.tile(iw, (8, 1))
        chunk = s_c >> 7
        erow = s_c & 127
        flat = erow * (CT * 256) + chunk * 256 + j_c
        ef = eqa[c].reshape(-1)
        ef[flat] = np.float32(1.0).astype(NPF8)
        ewn_pm[c][erow, chunk] = wv_c

    # ---- per-core node-indexed tensors ----
    real = row_node >= 0
    nid = np.maximum(row_node, 0)
    xw = np.where(real[:, None], x[nid], 0.0).astype(NPBF)   # [K*NPC, D]
    # x_t: [K, 64, NPC] columns in w-major local order (col = dloc)
    x_t = xw.reshape(K, NPC, D).transpose(0, 2, 1).copy()

    d2 = np.where(real, dis2[nid], 0.0).astype(np.float32)
    # dis2_pm [K, 128, W]: [p, w] = dis2 of dloc w*128+p
    dis2_pm = d2.reshape(K, W, 128).transpose(0, 2, 1).copy()

    bat = np.where(real, batch[nid], -1)
    eqp = np.zeros((K, 128, W * 256), dtype=NPF8)
    bkw = bat.reshape(K, W, 128)
    for c in range(K):
        p_i, w_i = np.meshgrid(np.arange(128), np.arange(W), indexing="ij")
        g = bkw[c].T                                    # [128, W]
        valid = g >= 0
        flat = (p_i * (W * 256) + w_i * 256 + g)[valid]
        ef = eqp[c].reshape(-1)
        ef[flat] = np.float32(1.0).astype(NPF8)

    counts = np.bincount(batch, minlength=cfg.G).astype(np.float32)
    data = {"x_t": x_t, "idxw": idxw, "eqa": eqa, "ewn": ewn_pm,
            "eqp": eqp, "dis2": dis2_pm}
    return data, counts, tuple(int(v) for v in Cs)


def build_nc(cfg, Cs, debug=False, act=AF.Gelu):
    K, W, NPC, D, GW = cfg.K, cfg.W, cfg.NPC, cfg.D, cfg.GW
    NP, PPC, BCH, SGRP = cfg.NP, cfg.PPC, cfg.BCH, cfg.SGRP
    off = [0]
    for c in Cs:
        off.append(off[-1] + c)
    CT = off[-1]
    # chunk -> pair, first/last flags
    cpair = np.empty(CT, dtype=np.int64)
    cfirst = np.zeros(CT, dtype=bool)
    clast = np.zeros(CT, dtype=bool)
    for p in range(NP):
        cpair[off[p]:off[p + 1]] = p
        cfirst[off[p]] = True
        clast[off[p + 1] - 1] = True
    batches = [(lo, min(lo + BCH, CT)) for lo in range(0, CT, BCH)]

    nc = bacc.Bacc("TRN2", target_bir_lowering=False, debug=debug)

    xt_d = nc.dram_tensor("x_t", [D, NPC], BF16, kind="ExternalInput")
    idx_d = nc.dram_tensor("idxw", [128, CT * 8], I16, kind="ExternalInput")
    eqa_d = nc.dram_tensor("eqa", [128, CT * 256], FP8, kind="ExternalInput")
    ewn_d = nc.dram_tensor("ewn", [128, CT], F32, kind="ExternalInput")
    eqp_d = nc.dram_tensor("eqp", [128, W * 256], FP8, kind="ExternalInput")
    dis2_d = nc.dram_tensor("dis2", [128, W], F32, kind="ExternalInput")
    w_d = [nc.dram_tensor(f"w{L}", [D, D], BF16, kind="ExternalInput")
           for L in (0, 1)]
    b_d = [nc.dram_tensor(f"b{L}b", [128, D], BF16, kind="ExternalInput")
           for L in (0, 1)]
    pool_out = [nc.dram_tensor(f"pool{L}", [GW * 128, D], F32,
                               kind="ExternalOutput") for L in (0, 1)]

    rg = [list(range(K))]

    with tile.TileContext(nc) as tc:
        with tc.tile_pool(name="const", bufs=1) as cpool, \
             tc.tile_pool(name="state", bufs=1) as spool, \
             tc.tile_pool(name="lhsT_p", bufs=1) as lhsT_p, \
             tc.tile_pool(name="dram", bufs=1, space="DRAM") as dpool, \
             tc.tile_pool(name="gath_p", bufs=3) as gath_p, \
             tc.tile_pool(name="eqa_p", bufs=9) as eqa_p, \
             tc.tile_pool(name="stage_p", bufs=3) as stage_p, \
             tc.tile_pool(name="ps_t", bufs=2, space="PSUM") as ps_t, \
             tc.tile_pool(name="ps_pair", bufs=4, space="PSUM") as ps_pair, \
             tc.tile_pool(name="ps_pool", bufs=2, space="PSUM") as ps_pool:

            nc.gpsimd.load_library(mlp)

            # consts
            wt = []
            for L in (0, 1):
                t = cpool.tile([D, D], BF16, name=f"wt{L}")
                nc.sync.dma_start(t[:], w_d[L][:])
                wt.append(t)
            bt = []
            for L in (0, 1):
                t = cpool.tile([128, D], BF16, name=f"bt{L}")
                nc.sync.dma_start(t[:], b_d[L][:])
                bt.append(t)
            dis2_sb = cpool.tile([128, W], F32, name="dis2_sb")
            nc.scalar.dma_start(dis2_sb[:], dis2_d[:])
            eqp_sb = cpool.tile([128, W * 256], FP8, name="eqp_sb")
            nc.scalar.dma_start(eqp_sb[:], eqp_d[:])
            idx_sb = cpool.tile([128, CT * 8], I16, name="idx_sb")
            nc.sync.dma_start(idx_sb[:], idx_d[:])
            ewn_sb = cpool.tile([128, CT], F32, name="ewn_sb")
            nc.sync.dma_start(ewn_sb[:], ewn_d[:])

            t_sb = spool.tile([128, W * 128], BF16, name="t_sb")
            nc.vector.memset(t_sb[:], 0)      # pad halves stay zero
            h_sb = spool.tile([128, W * D], BF16, name="h_sb")   # scratch
            rs_sb = spool.tile([128, W * D], BF16, name="rs_sb")  # rs, then h

            table_d = dpool.tile([NPC, 128], BF16, name="table")
            parts_d = dpool.tile([K * NPC, D], BF16, name="parts")
            rs_out_d = dpool.tile([NPC, D], BF16, name="rs_out")
            h0_d = dpool.tile([NPC, D], BF16, name="h0")

            xt_sb = lhsT_p.tile([D, NPC], BF16, name="xt_sb")
            nc.scalar.dma_start(xt_sb[:], xt_d[:])

            # partials DRAM view: [k][p][q=(pair_local*2+w01)][64]
            parts_v = parts_d[:].rearrange("(k p q) e -> k p (q e)",
                                           k=K, p=128)
            # rs_out rows r=p*W+w -> per-partition contiguous
            rs_v = rs_out_d[:].rearrange("(p r) e -> p (r e)", p=128)
            # h0 rows d = w*128+p
            h0_v = h0_d[:].rearrange("(w p) e -> p w e", p=128)
            # table rows r = p*W+w
            tab_v = table_d[:].rearrange("(p r) e -> p (r e)", p=128)

            t3 = t_sb[:].rearrange("p (w e) -> p w e", e=128)
            h3 = h_sb[:].rearrange("p (w e) -> p w e", e=D)
            rs3 = rs_sb[:].rearrange("p (w e) -> p w e", e=D)

            hT_sb = None

            def t_phase(L, lhsT):
                # t = h @ W  (8 windows per PSUM bank)
                for wb in range(0, W, 8):
                    nwin = min(8, W - wb)
                    pt = ps_t.tile([128, nwin * D], F32, name="pt",
                                   space="PSUM")
                    for i in range(nwin):
                        w = wb + i
                        # one zero-region: first mm starts, last stops
                        nc.tensor.matmul(
                            pt[:, i * D:(i + 1) * D],
                            lhsT=lhsT[:, w * 128:(w + 1) * 128],
                            rhs=wt[L][:], start=(i == 0),
                            stop=(i == nwin - 1))
                    dst = t3[:, wb:wb + nwin, 0:D]
                    src = pt[:].rearrange("p (w e) -> p w e", e=D)
                    eng = nc.vector if (wb // 8) % 2 == 0 else nc.scalar
                    if eng is nc.vector:
                        eng.tensor_copy(dst, src)
                    else:
                        eng.copy(dst, src)
                # scalar queue: keep SP free for stream prefetch run-ahead
                nc.scalar.dma_start(tab_v, t_sb[:])

            def stream(L):
                pair_ps = {}
                for (lo, hi) in batches:
                    nch = hi - lo
                    gath = gath_p.tile([128, BCH * 128], BF16, name="gath")
                    g3 = gath[:].rearrange("p (c e) -> p c e", e=128)
                    nc.gpsimd.dma_gather(
                        g3[:, 0:nch, :],
                        table_d[:], idx_sb[:, lo * 8:hi * 8],
                        nch * 128, nch * 128, 128, single_packet=False)
                    # scale the gathered rows by the f32 edge norm in place
                    wb_ = ewn_sb[:, lo:hi].unsqueeze(2).broadcast_to(
                        (128, nch, D))
                    nc.vector.tensor_tensor(out=g3[:, 0:nch, 0:D],
                                            in0=g3[:, 0:nch, 0:D],
                                            in1=wb_, op=OP.mult)
                    eqa_t = eqa_p.tile([128, BCH * 256], FP8, name="eqa_t")
                    nc.sync.dma_start(eqa_t[:, 0:nch * 256],
                                      eqa_d[:, lo * 256:hi * 256])
                    for c in range(lo, hi):
                        pr = int(cpair[c])
                        if cfirst[c]:
                            pair_ps[pr] = ps_pair.tile(
                                [128, 2 * D], F32, name="pp",
                                space="PSUM")
                        pp = pair_ps[pr]
                        cb = (c - lo) * 256
                        rhs = gath[:, (c - lo) * 128:(c - lo) * 128 + D]
                        # both windows share one psum zero-region: only the
                        # pair's very first mm starts it, the last stops it
                        nc.tensor.matmul(pp[:, 0:D],
                                         lhsT=eqa_t[:, cb:cb + 128],
                                         rhs=rhs, start=bool(cfirst[c]),
                                         stop=False)
                        nc.tensor.matmul(pp[:, D:2 * D],
                                         lhsT=eqa_t[:, cb + 128:cb + 256],
                                         rhs=rhs, start=False,
                                         stop=bool(clast[c]))
                        if clast[c]:
                            g = pr // SGRP
                            gslot = pr % SGRP
                            if gslot == 0:
                                stage_t = stage_p.tile(
                                    [128, SGRP * 2 * D], BF16, name="stage")
                                pair_ps["stage"] = stage_t
                            stage_t = pair_ps["stage"]
                            dstp = stage_t[:, gslot * 2 * D:(gslot + 1) * 2 * D]
                            if pr % 2 == 0:
                                nc.vector.tensor_copy(dstp, pp[:])
                            else:
                                nc.scalar.copy(dstp, pp[:])
                            del pair_ps[pr]
                            if gslot == SGRP - 1:
                                kd = pr // PPC
                                pl0 = (g % (PPC // SGRP)) * SGRP
                                nc.sync.dma_start(
                                    parts_v[kd][:, pl0 * 2 * D:
                                                (pl0 + SGRP) * 2 * D],
                                    stage_t[:])

            def post(L):
                nonlocal hT_sb
                nc.gpsimd.collective_compute(
                    "ReduceScatter", OP.add,
                    ins=[parts_d[:]], outs=[rs_out_d[:]], replica_groups=rg)
                # scalar queue: the rs load waits on the collective; on SP it
                # would head-of-line-block the next layer's eqa prefetch
                nc.scalar.dma_start(rs_sb[:], rs_v)
                # h = gelu(rs + dis2 * t + b); h_sb is scratch, the final
                # activations land in rs_sb (rs no longer needed then).
                d2b = dis2_sb[:].unsqueeze(2).broadcast_to((128, W, D))
                nc.vector.tensor_tensor(out=h3, in0=t3[:, :, 0:D],
                                        in1=d2b, op=OP.mult)
                nc.vector.tensor_tensor(out=h3, in0=h3, in1=rs3, op=OP.add)
                bb = bt[L][:].unsqueeze(1).broadcast_to((128, W, D))
                nc.vector.tensor_tensor(out=h3, in0=h3, in1=bb, op=OP.add)
                nc.scalar.activation(rs_sb[:], h_sb[:], act)
                # pooling (h lives in rs_sb/rs3 now)
                pps = ps_pool.tile([128, GW * D], F32, name="pool_ps",
                                   space="PSUM")
                for w in range(W):
                    for gw in range(GW):
                        nc.tensor.matmul(
                            pps[:, gw * D:(gw + 1) * D],
                            lhsT=eqp_sb[:, w * 256 + gw * 128:
                                        w * 256 + gw * 128 + 128],
                            rhs=rs3[:, w, :],
                            start=(w == 0 and gw == 0),
                            stop=(w == W - 1 and gw == GW - 1))
                pk = stage_p.tile([128, GW * D], F32, name="pk")
                nc.scalar.copy(pk[:], pps[:])
                nc.scalar.dma_start(
                    pool_out[L][:].rearrange("(g r) e -> r g e", g=GW),
                    pk[:].rearrange("p (g e) -> p g e", g=GW))
                if L == 0:
                    nc.scalar.dma_start(h0_v, rs_sb[:])
                    hT_sb = lhsT_p.tile([D, NPC], BF16, name="hT_sb")
                    nc.scalar.dma_start_transpose(hT_sb[:], h0_d[:])

            # ---- program ----
            t_phase(0, xt_sb[:])
            stream(0)
            post(0)
            t_phase(1, hT_sb[:])
            stream(1)
            post(1)

    nc.finalize()
    return nc


_NC_CACHE = {}


def get_nc(cfg, Cs, act=AF.Gelu):
    key = (cfg.N, cfg.E, cfg.G, cfg.K, Cs, act)
    if key not in _NC_CACHE:
        _NC_CACHE[key] = build_nc(cfg, Cs, act=act)
    return _NC_CACHE[key]


def make_in_maps(cfg, data, W0, b0, W1, b1):
    D = cfg.D
    w0 = np.asarray(W0, np.float32).astype(NPBF)
    w1 = np.asarray(W1, np.float32).astype(NPBF)
    b0b = np.ascontiguousarray(np.broadcast_to(
        np.asarray(b0, np.float32).astype(NPBF), (128, D)))
    b1b = np.ascontiguousarray(np.broadcast_to(
        np.asarray(b1, np.float32).astype(NPBF), (128, D)))
    maps = []
    for k in range(cfg.K):
        m = {name: arr[k] for name, arr in data.items()}
        m.update({"w0": w0, "w1": w1, "b0b": b0b, "b1b": b1b})
        maps.append(m)
    return maps


def postprocess(cfg, results, counts):
    outs = []
    denom = np.maximum(counts, 1.0).astype(np.float32)
    for L in (0, 1):
        tot = np.zeros((cfg.GW * 128, cfg.D), dtype=np.float32)
        for k in range(cfg.K):
            tot += results[k][f"pool{L}"]
        outs.append((tot[:cfg.G] / denom[:, None]).astype(np.float32))
    return tuple(outs)


def kernel(x, edge_index, edge_weight, batch, W0, b0, W1, b1):
    cfg = FULL
    data, counts, Cs = prep_host(cfg, x, edge_index, edge_weight, batch)
    nc = get_nc(cfg, Cs)
    in_maps = make_in_maps(cfg, data, W0, b0, W1, b1)
    res = run_bass_kernel_spmd(nc, in_maps, list(range(cfg.K)))
    return postprocess(cfg, res.results, counts)
